# revision 9
# baseline (speedup 1.0000x reference)
"""Bayesian GPLVM collapsed-ELBO kernel for Trainium2 (8 NeuronCores).

Sharding: data-parallel over n (2048 rows -> 256 per core). Each core
computes its partial psi2 = sum_n exp(log_psi2_n) (m*m = 4096 entries),
partial A = psi1^T y (64x256), and partial row statistics (KL pieces,
sum y^2). Host sums the 8 partials and does the small m x m linear
algebra (Cholesky solves / slogdets) to produce the scalar ELBO.

Device layout per core (n_loc = 256, two 128-row chunks):
  - NPREP (98 x 256, q-major n-side): rows 0..15 = (q_mu*w1)^T,
    16..31 = w1^T, 32 = h1, 64..79 = (q_mu*w2)^T, 80..95 = w2^T,
    96 = g, 97 = ones (matmul operands need base partition in
    {0,32,64}, so the psi2 block sits at 64). Built n-major as a
    (128 x 98) tile per chunk, then PE-transposed.
  - psi1 exponent = NPREP[0:33,chunk]^T @ ZS1 (z-side, host-built),
    one matmul + Exp per chunk; A accumulates psi1^T y in PSUM.
  - psi2 exponent for each of 32 ij-chunks (128 ij-pairs each) =
    ZL[64:98, chunk]^T @ NPREP[64:98]; Exp with fused free-axis
    accumulation gives the local n-sum directly.
"""

import numpy as np

N, D, Q, M = 2048, 256, 16, 64
NCORES = 8
NLOC = N // NCORES          # 256
F32 = None                  # set lazily (mybir.dt.float32)

_compiled = None


def _build_bass():
    import concourse.bacc as bacc
    import concourse.bass as bass  # noqa: F401
    import concourse.mybir as mybir
    from concourse import masks
    from concourse.tile import TileContext

    f32 = mybir.dt.float32
    AF = mybir.ActivationFunctionType
    OP = mybir.AluOpType

    nc = bacc.Bacc("TRN2", target_bir_lowering=False)

    y_d = nc.declare_dram_parameter("y", [NLOC, D], f32, isOutput=False)
    qmu_d = nc.declare_dram_parameter("qmu", [NLOC, Q], f32, isOutput=False)
    qls_d = nc.declare_dram_parameter("qls", [NLOC, Q], f32, isOutput=False)
    zl_d = nc.declare_dram_parameter("zl", [34, M * M], f32, isOutput=False)
    zs1_d = nc.declare_dram_parameter("zs1", [33, M], f32, isOutput=False)
    alpha_d = nc.declare_dram_parameter("alpha", [128, Q], f32, isOutput=False)
    consts_d = nc.declare_dram_parameter("consts", [128, 4], f32, isOutput=False)
    psi2_o = nc.declare_dram_parameter("out_psi2", [128, 32], f32, isOutput=True)
    a_o = nc.declare_dram_parameter("out_A", [M, D], f32, isOutput=True)
    misc_o = nc.declare_dram_parameter("out_misc", [128, 8], f32, isOutput=True)

    with TileContext(nc) as tc:
        with (
            tc.tile_pool(name="const", bufs=1) as cpool,
            tc.tile_pool(name="big", bufs=1) as bigpool,
            tc.tile_pool(name="work", bufs=3) as wpool,
            tc.tile_pool(name="scr", bufs=3) as spool,
            tc.tile_pool(name="psum", bufs=2, space="PSUM") as ppool,
            tc.tile_pool(name="psum1", bufs=1, space="PSUM") as ppool1,
        ):
            ident = cpool.tile([128, 128], f32)
            masks.make_identity(nc, ident[:])

            alpha_b = cpool.tile([128, Q], f32)
            nc.sync.dma_start(out=alpha_b[:, :], in_=alpha_d[:, :])

            consts_b = cpool.tile([128, 4], f32)
            nc.sync.dma_start(out=consts_b[:, :], in_=consts_d[:, :])

            zl_sb = bigpool.tile([98, M * M], f32)
            nc.sync.dma_start(out=zl_sb[64:98, :], in_=zl_d[:, :])
            zs1_sb = cpool.tile([33, M], f32)
            nc.sync.dma_start(out=zs1_sb[:, :], in_=zs1_d[:, :])

            nprep = bigpool.tile([98, 2 * 128], f32)
            stats = bigpool.tile([128, 32], f32)
            misc = bigpool.tile([128, 8], f32)
            apsum = ppool1.tile([M, D], f32)

            for c in range(2):
                r0, r1 = c * 128, (c + 1) * 128
                qm = wpool.tile([128, Q], f32, tag="qm")
                nc.sync.dma_start(out=qm[:, :], in_=qmu_d[r0:r1, :])
                qls = wpool.tile([128, Q], f32, tag="qls")
                nc.sync.dma_start(out=qls[:, :], in_=qls_d[r0:r1, :])
                yc = wpool.tile([128, D], f32, tag="yc")
                nc.sync.dma_start(out=yc[:, :], in_=y_d[r0:r1, :])

                prepn = wpool.tile([128, 98], f32, tag="prepn")
                qsig = wpool.tile([128, Q], f32, tag="qsig")
                d1 = wpool.tile([128, Q], f32, tag="d1")
                d2 = wpool.tile([128, Q], f32, tag="d2")
                rcp = wpool.tile([128, Q], f32, tag="rcp")
                scr16 = spool.tile([128, Q], f32, tag="scr16")
                scrY = spool.tile([128, D], f32, tag="scrY")
                cols = wpool.tile([128, 8], f32, tag="cols")
                sum2c = cols[:, 0:1]
                s3x2c = cols[:, 1:2]
                rt1c = cols[:, 2:3]
                ac = cols[:, 3:4]
                t1c = cols[:, 4:5]
                t2c = cols[:, 5:6]

                # q_sigma = softplus(qls) = ln(1 + exp(qls))
                nc.scalar.activation(scr16[:, :], qls[:, :], AF.Exp)
                nc.scalar.activation(qsig[:, :], scr16[:, :], AF.Ln, bias=1.0)
                nc.vector.tensor_mul(d1[:, :], qsig[:, :], alpha_b[:, :])
                nc.vector.tensor_scalar_add(d1[:, :], d1[:, :], 1.0)
                # w1 = alpha / d1
                nc.vector.reciprocal(rcp[:, :], d1[:, :])
                nc.vector.tensor_mul(prepn[:, 16:32], rcp[:, :], alpha_b[:, :])
                # sum2 = sum_q log d1
                nc.scalar.activation(scr16[:, :], d1[:, :], AF.Ln, accum_out=sum2c)
                # d2 = 2*d1 - 1;  w2 = alpha / d2
                nc.scalar.activation(d2[:, :], d1[:, :], AF.Copy, scale=2.0, bias=-1.0)
                nc.vector.reciprocal(rcp[:, :], d2[:, :])
                nc.vector.tensor_mul(prepn[:, 80:96], rcp[:, :], alpha_b[:, :])
                # 2*s3 = sum_q log d2
                nc.scalar.activation(scr16[:, :], d2[:, :], AF.Ln, accum_out=s3x2c)
                # q_mu * w1, q_mu * w2
                nc.vector.tensor_mul(prepn[:, 0:16], qm[:, :], prepn[:, 16:32])
                nc.vector.tensor_mul(prepn[:, 64:80], qm[:, :], prepn[:, 80:96])
                # rt1 = sum_q q_mu^2 w1 ; a = sum_q q_mu^2 w2
                nc.vector.tensor_mul(scr16[:, :], prepn[:, 0:16], qm[:, :])
                nc.vector.tensor_reduce(rt1c, scr16[:, :],
                                        axis=mybir.AxisListType.X, op=OP.add)
                nc.vector.tensor_mul(scr16[:, :], prepn[:, 64:80], qm[:, :])
                nc.vector.tensor_reduce(ac, scr16[:, :],
                                        axis=mybir.AxisListType.X, op=OP.add)
                # h1 = 2*logvar - 0.5*(rt1 + sum2)
                nc.vector.tensor_add(t1c, rt1c, sum2c)
                nc.vector.tensor_scalar(
                    out=prepn[:, 32:33], in0=t1c, scalar1=-0.5,
                    scalar2=consts_b[:, 0:1], op0=OP.mult, op1=OP.add)
                # g = 4*logvar - 0.5*(2*s3) - a
                nc.vector.tensor_scalar(
                    out=t2c, in0=s3x2c, scalar1=0.5, scalar2=ac,
                    op0=OP.mult, op1=OP.add)
                nc.vector.tensor_scalar(
                    out=prepn[:, 96:97], in0=t2c, scalar1=-1.0,
                    scalar2=consts_b[:, 1:2], op0=OP.mult, op1=OP.add)
                nc.vector.memset(prepn[:, 97:98], 1.0)
                nc.vector.memset(prepn[:, 33:64], 0.0)

                # KL / trace statistics
                nc.scalar.activation(scr16[:, :], qsig[:, :], AF.Ln,
                                     accum_out=misc[:, 0 + c:1 + c])
                nc.scalar.activation(scr16[:, :], qsig[:, :], AF.Square,
                                     accum_out=misc[:, 2 + c:3 + c])
                nc.scalar.activation(scr16[:, :], qm[:, :], AF.Square,
                                     accum_out=misc[:, 4 + c:5 + c])
                nc.scalar.activation(scrY[:, :], yc[:, :], AF.Square,
                                     accum_out=misc[:, 6 + c:7 + c])

                # transpose prep (128 x 67) -> NPREP[:, chunk]
                ptp = ppool.tile([98, 128], f32, tag="ptp")
                nc.tensor.transpose(ptp[:, :], prepn[:, :], ident[:, :])
                nc.vector.tensor_copy(nprep[:, r0:r1], ptp[:, :])

                # psi1 chunk: exponent (128 n x 64 m) then exp
                e1 = ppool.tile([128, M], f32, tag="e1")
                nc.tensor.matmul(e1[:, :], lhsT=nprep[0:33, r0:r1],
                                 rhs=zs1_sb[:, :], start=True, stop=True)
                psi1c = wpool.tile([128, M], f32, tag="psi1c")
                nc.scalar.activation(psi1c[:, :], e1[:, :], AF.Exp)
                # A += psi1_c^T @ y_c
                nc.tensor.matmul(apsum[:, :], lhsT=psi1c[:, :], rhs=yc[:, :],
                                 start=(c == 0), stop=(c == 1))

            # psi2: 32 ij-chunks of 128 pairs
            for ch in range(32):
                p2 = ppool.tile([128, NLOC], f32, tag="p2")
                nc.tensor.matmul(p2[:, :],
                                 lhsT=zl_sb[64:98, ch * 128:(ch + 1) * 128],
                                 rhs=nprep[64:98, :], start=True, stop=True)
                scr = spool.tile([128, NLOC], f32, tag="p2scr")
                nc.scalar.activation(scr[:, :], p2[:, :], AF.Exp,
                                     accum_out=stats[:, ch:ch + 1])

            a_sb = bigpool.tile([M, D], f32)
            nc.vector.tensor_copy(a_sb[:, :], apsum[:, :])
            nc.sync.dma_start(out=psi2_o[:, :], in_=stats[:, :])
            nc.sync.dma_start(out=a_o[:, :], in_=a_sb[:, :])
            nc.sync.dma_start(out=misc_o[:, :], in_=misc[:, :])

    nc.compile()
    return nc


def _get_compiled():
    global _compiled
    if _compiled is None:
        _compiled = _build_bass()
    return _compiled


def _np_softplus(x):
    return np.logaddexp(x, 0.0)


def kernel(y, q_mu, q_log_sigma, z, noise_raw, alpha, variance, _trace=False):
    from concourse.bass_utils import run_bass_kernel_spmd

    nc = _get_compiled()

    f8 = np.float64
    z64 = z.astype(f8)
    al = alpha.astype(f8)
    var = f8(variance[0])
    logvar = np.log(var)

    # z-side stationary blocks (host-built, replicated to all cores)
    S = z64[:, None, :] + z64[None, :, :]                  # (m, m, q)
    zl = np.empty((34, M * M), np.float32)
    zl[0:16] = S.transpose(2, 0, 1).reshape(Q, M * M)
    zl[16:32] = (-0.25 * S * S).transpose(2, 0, 1).reshape(Q, M * M)
    zl[32] = 1.0
    sqz = (z64[:, None, :] - z64[None, :, :]) ** 2          # (m, m, q)
    s1 = 0.25 * (sqz @ al)                                  # (m, m)
    zl[33] = (-s1).reshape(M * M)

    zt = z64.T                                              # (q, m)
    zs1 = np.empty((33, M), np.float32)
    zs1[0:16] = zt
    zs1[16:32] = -0.5 * zt * zt
    zs1[32] = 1.0

    consts = np.tile(np.array([[2.0 * logvar, 4.0 * logvar, 0.0, 0.0]],
                              np.float32), (128, 1))
    alpha_in = np.tile(alpha.reshape(1, Q).astype(np.float32), (128, 1))

    in_maps = []
    for i in range(NCORES):
        sl = slice(i * NLOC, (i + 1) * NLOC)
        in_maps.append({
            "y": np.ascontiguousarray(y[sl], dtype=np.float32),
            "qmu": np.ascontiguousarray(q_mu[sl], dtype=np.float32),
            "qls": np.ascontiguousarray(q_log_sigma[sl], dtype=np.float32),
            "zl": zl,
            "zs1": zs1,
            "alpha": alpha_in,
            "consts": consts,
        })

    br = run_bass_kernel_spmd(nc, in_maps, list(range(NCORES)), trace=_trace)
    res = br.results

    psi2_part = np.zeros((128, 32), f8)
    A = np.zeros((M, D), f8)
    misc = np.zeros(8, f8)
    for r in res:
        psi2_part += r["out_psi2"].astype(f8)
        A += r["out_A"].astype(f8)
        misc += r["out_misc"].astype(f8).sum(axis=0)

    psi2 = psi2_part.T.reshape(M * M)[: M * M].reshape(M, M)
    lnsig = misc[0] + misc[1]
    ssq = misc[2] + misc[3]
    musq = misc[4] + misc[5]
    tr_yy = misc[6] + misc[7]

    kl_sum = -lnsig + 0.5 * (ssq + musq) - 0.5 * N * Q
    kl_term = kl_sum / (N * D)

    # small m x m algebra on host
    k_mm = var * np.exp(-0.5 * (sqz @ al))                  # (m, m)
    noise_var = _np_softplus(f8(noise_raw[0]))
    beta = 1.0 / noise_var
    psi0 = N * var

    cov1 = beta * psi2 + k_mm
    B = np.linalg.solve(cov1, A)
    tr_yWy = beta * tr_yy - np.sum(A * B)

    F = 0.5 * N * np.log(beta)
    F += 0.5 * np.linalg.slogdet(k_mm)[1]
    F -= 0.5 * N * np.log(np.pi)
    F -= 0.5 * np.linalg.slogdet(cov1)[1]
    F -= 0.5 * beta * psi0
    F += 0.5 * np.trace(np.linalg.solve(k_mm, psi2))
    F = (F * D - 0.5 * tr_yWy) / (N * D)

    out = F - kl_term
    result = np.asarray(out, dtype=np.float32)
    if _trace:
        return result, br
    return result


# revision 12
# speedup vs baseline: 1.5802x; 1.5802x over previous
"""Bayesian GPLVM collapsed-ELBO kernel for Trainium2 (8 NeuronCores).

Sharding: data-parallel over n (2048 rows -> 256 per core). Each core
computes its partial psi2 = sum_n exp(log_psi2_n) (m*m = 4096 entries),
partial A = psi1^T y (64x256), and partial row statistics (KL pieces,
sum y^2). Host sums the 8 partials and does the small m x m linear
algebra (Cholesky solves / slogdets) to produce the scalar ELBO.

Device layout per core (n_loc = 256, two 128-row chunks):
  - NPREP (98 x 256, q-major n-side): rows 0..15 = (q_mu*w1)^T,
    16..31 = w1^T, 32 = h1, 64..79 = (q_mu*w2)^T, 80..95 = w2^T,
    96 = g, 97 = ones (matmul operands need base partition in
    {0,32,64}, so the psi2 block sits at 64). Built n-major as a
    (128 x 98) tile per chunk, then PE-transposed.
  - psi1 exponent = NPREP[0:33,chunk]^T @ ZS1 (z-side, host-built),
    one matmul + Exp per chunk; A accumulates psi1^T y in PSUM.
  - psi2 exponent for each of 32 ij-chunks (128 ij-pairs each) =
    ZL[64:98, chunk]^T @ NPREP[64:98]; Exp with fused free-axis
    accumulation gives the local n-sum directly.
"""

import numpy as np

N, D, Q, M = 2048, 256, 16, 64
NCORES = 8
NLOC = N // NCORES          # 256
F32 = None                  # set lazily (mybir.dt.float32)

_compiled = None


def _build_bass():
    import concourse.bacc as bacc
    import concourse.bass as bass  # noqa: F401
    import concourse.mybir as mybir
    from concourse import masks
    from concourse.tile import TileContext

    f32 = mybir.dt.float32
    f32r = mybir.dt.float32r
    AF = mybir.ActivationFunctionType
    OP = mybir.AluOpType

    nc = bacc.Bacc("TRN2", target_bir_lowering=False)

    y_d = nc.declare_dram_parameter("y", [NLOC, D], f32r, isOutput=False)
    qmu_d = nc.declare_dram_parameter("qmu", [NLOC, Q], f32, isOutput=False)
    qls_d = nc.declare_dram_parameter("qls", [NLOC, Q], f32, isOutput=False)
    zl_d = nc.declare_dram_parameter("zl", [34, M * M], f32r, isOutput=False)
    zs1_d = nc.declare_dram_parameter("zs1", [33, M], f32r, isOutput=False)
    alpha_d = nc.declare_dram_parameter("alpha", [128, Q], f32, isOutput=False)
    consts_d = nc.declare_dram_parameter("consts", [128, 4], f32, isOutput=False)
    psi2_o = nc.declare_dram_parameter("out_psi2", [128, 32], f32, isOutput=True)
    a_o = nc.declare_dram_parameter("out_A", [M, D], f32, isOutput=True)
    misc_o = nc.declare_dram_parameter("out_misc", [128, 8], f32, isOutput=True)

    with TileContext(nc) as tc:
        with (
            tc.tile_pool(name="const", bufs=1) as cpool,
            tc.tile_pool(name="big", bufs=1) as bigpool,
            tc.tile_pool(name="work", bufs=3) as wpool,
            tc.tile_pool(name="scr", bufs=3) as spool,
            tc.tile_pool(name="psum", bufs=2, space="PSUM") as ppool,
            tc.tile_pool(name="psums", bufs=1, space="PSUM") as ppools,
            tc.tile_pool(name="psum1", bufs=1, space="PSUM") as ppool1,
        ):
            ident = cpool.tile([128, 128], f32)
            masks.make_identity(nc, ident[:])

            alpha_b = cpool.tile([128, Q], f32)
            nc.sync.dma_start(out=alpha_b[:, :], in_=alpha_d[:, :])

            consts_b = cpool.tile([128, 4], f32)
            nc.sync.dma_start(out=consts_b[:, :], in_=consts_d[:, :])

            zl_sb = bigpool.tile([98, M * M], f32r)
            nc.sync.dma_start(out=zl_sb[64:98, :], in_=zl_d[:, :])
            zs1_sb = cpool.tile([33, M], f32r)
            nc.sync.dma_start(out=zs1_sb[:, :], in_=zs1_d[:, :])

            nprep = bigpool.tile([98, 2 * 128], f32r)
            stats = bigpool.tile([128, 32], f32)
            misc = bigpool.tile([128, 8], f32)
            apsum = ppool1.tile([M, D], f32)

            for c in range(2):
                r0, r1 = c * 128, (c + 1) * 128
                qm = wpool.tile([128, Q], f32, tag="qm")
                nc.sync.dma_start(out=qm[:, :], in_=qmu_d[r0:r1, :])
                qls = wpool.tile([128, Q], f32, tag="qls")
                nc.sync.dma_start(out=qls[:, :], in_=qls_d[r0:r1, :])
                yc = wpool.tile([128, D], f32r, tag="yc")
                nc.sync.dma_start(out=yc[:, :], in_=y_d[r0:r1, :])

                prepn = wpool.tile([128, 98], f32, tag="prepn")
                qsig = wpool.tile([128, Q], f32, tag="qsig")
                d1 = wpool.tile([128, Q], f32, tag="d1")
                d2 = wpool.tile([128, Q], f32, tag="d2")
                rcp = wpool.tile([128, Q], f32, tag="rcp")
                scr16 = spool.tile([128, Q], f32, tag="scr16")
                scrY = spool.tile([128, D], f32, tag="scrY")
                cols = wpool.tile([128, 8], f32, tag="cols")
                sum2c = cols[:, 0:1]
                s3x2c = cols[:, 1:2]
                rt1c = cols[:, 2:3]
                ac = cols[:, 3:4]
                t1c = cols[:, 4:5]
                t2c = cols[:, 5:6]

                # q_sigma = softplus(qls) = ln(1 + exp(qls))
                nc.scalar.activation(scr16[:, :], qls[:, :], AF.Exp)
                nc.scalar.activation(qsig[:, :], scr16[:, :], AF.Ln, bias=1.0)
                nc.vector.tensor_mul(d1[:, :], qsig[:, :], alpha_b[:, :])
                nc.vector.tensor_scalar_add(d1[:, :], d1[:, :], 1.0)
                # w1 = alpha / d1
                nc.vector.reciprocal(rcp[:, :], d1[:, :])
                nc.vector.tensor_mul(prepn[:, 16:32], rcp[:, :], alpha_b[:, :])
                # sum2 = sum_q log d1
                nc.scalar.activation(scr16[:, :], d1[:, :], AF.Ln, accum_out=sum2c)
                # d2 = 2*d1 - 1;  w2 = alpha / d2
                nc.vector.tensor_scalar(
                    out=d2[:, :], in0=d1[:, :], scalar1=2.0, scalar2=-1.0,
                    op0=OP.mult, op1=OP.add)
                nc.vector.reciprocal(rcp[:, :], d2[:, :])
                nc.vector.tensor_mul(prepn[:, 80:96], rcp[:, :], alpha_b[:, :])
                # 2*s3 = sum_q log d2
                nc.scalar.activation(scr16[:, :], d2[:, :], AF.Ln, accum_out=s3x2c)
                # q_mu * w1, q_mu * w2
                nc.vector.tensor_mul(prepn[:, 0:16], qm[:, :], prepn[:, 16:32])
                nc.vector.tensor_mul(prepn[:, 64:80], qm[:, :], prepn[:, 80:96])
                # rt1 = sum_q q_mu^2 w1 ; a = sum_q q_mu^2 w2
                nc.vector.tensor_mul(scr16[:, :], prepn[:, 0:16], qm[:, :])
                nc.vector.tensor_reduce(rt1c, scr16[:, :],
                                        axis=mybir.AxisListType.X, op=OP.add)
                nc.vector.tensor_mul(scr16[:, :], prepn[:, 64:80], qm[:, :])
                nc.vector.tensor_reduce(ac, scr16[:, :],
                                        axis=mybir.AxisListType.X, op=OP.add)
                # h1 = 2*logvar - 0.5*(rt1 + sum2)
                nc.vector.tensor_add(t1c, rt1c, sum2c)
                nc.vector.tensor_scalar(
                    out=prepn[:, 32:33], in0=t1c, scalar1=-0.5,
                    scalar2=consts_b[:, 0:1], op0=OP.mult, op1=OP.add)
                # g = 4*logvar - 0.5*(2*s3) - a
                nc.vector.tensor_scalar(
                    out=t2c, in0=s3x2c, scalar1=0.5, scalar2=ac,
                    op0=OP.mult, op1=OP.add)
                nc.vector.tensor_scalar(
                    out=prepn[:, 96:97], in0=t2c, scalar1=-1.0,
                    scalar2=consts_b[:, 1:2], op0=OP.mult, op1=OP.add)
                nc.vector.memset(prepn[:, 97:98], 1.0)
                nc.vector.memset(prepn[:, 33:64], 0.0)

                # KL / trace statistics (squares on DVE, Ln stays on ACT)
                nc.scalar.activation(scr16[:, :], qsig[:, :], AF.Ln,
                                     accum_out=misc[:, 0 + c:1 + c])
                nc.vector.tensor_mul(scr16[:, :], qsig[:, :], qsig[:, :])
                nc.vector.tensor_reduce(misc[:, 2 + c:3 + c], scr16[:, :],
                                        axis=mybir.AxisListType.X, op=OP.add)
                nc.vector.tensor_mul(scr16[:, :], qm[:, :], qm[:, :])
                nc.vector.tensor_reduce(misc[:, 4 + c:5 + c], scr16[:, :],
                                        axis=mybir.AxisListType.X, op=OP.add)
                nc.vector.tensor_mul(scrY[:, :], yc[:, :].bitcast(f32), yc[:, :].bitcast(f32))
                nc.vector.tensor_reduce(misc[:, 6 + c:7 + c], scrY[:, :],
                                        axis=mybir.AxisListType.X, op=OP.add)

                # transpose prep (128 x 67) -> NPREP[:, chunk]
                ptp = ppools.tile([98, 128], f32, tag="ptp")
                nc.tensor.transpose(ptp[:, :], prepn[:, :], ident[:, :])
                nc.vector.tensor_copy(nprep[:, r0:r1], ptp[:, :])

                # psi1 chunk: exponent (128 n x 64 m) then exp
                e1 = ppools.tile([128, M], f32, tag="e1")
                nc.tensor.matmul(e1[:, :],
                                 lhsT=nprep[0:33, r0:r1],
                                 rhs=zs1_sb[:, :],
                                 start=True, stop=True)
                psi1c = wpool.tile([128, M], f32r, tag="psi1c")
                nc.scalar.activation(psi1c[:, :], e1[:, :], AF.Exp)
                # A += psi1_c^T @ y_c
                nc.tensor.matmul(apsum[:, :], lhsT=psi1c[:, :],
                                 rhs=yc[:, :],
                                 start=(c == 0), stop=(c == 1))

            # psi2: 32 ij-chunks of 128 pairs, 4 chunks per PSUM tile
            for t in range(8):
                p2 = ppool.tile([128, 4 * NLOC], f32, tag="p2")
                for j in range(4):
                    ch = 4 * t + j
                    nc.tensor.matmul(
                        p2[:, j * NLOC:(j + 1) * NLOC],
                        lhsT=zl_sb[64:98, ch * 128:(ch + 1) * 128],
                        rhs=nprep[64:98, :],
                        start=True, stop=True)
                scr = spool.tile([128, 4 * NLOC], f32, tag="p2scr")
                nc.scalar.activation(scr[:, :], p2[:, :], AF.Exp)
                nc.vector.tensor_reduce(
                    stats[:, 4 * t:4 * t + 4],
                    scr[:, :].rearrange("p (a b) -> p a b", b=NLOC),
                    axis=mybir.AxisListType.X, op=OP.add)

            a_sb = bigpool.tile([M, D], f32)
            nc.vector.tensor_copy(a_sb[:, :], apsum[:, :])
            nc.sync.dma_start(out=psi2_o[:, :], in_=stats[:, :])
            nc.sync.dma_start(out=a_o[:, :], in_=a_sb[:, :])
            nc.sync.dma_start(out=misc_o[:, :], in_=misc[:, :])

    nc.compile()
    return nc


def _get_compiled():
    global _compiled
    if _compiled is None:
        _compiled = _build_bass()
    return _compiled


def _np_softplus(x):
    return np.logaddexp(x, 0.0)


def kernel(y, q_mu, q_log_sigma, z, noise_raw, alpha, variance, _trace=False):
    from concourse.bass_utils import run_bass_kernel_spmd

    nc = _get_compiled()

    f8 = np.float64
    z64 = z.astype(f8)
    al = alpha.astype(f8)
    var = f8(variance[0])
    logvar = np.log(var)

    # z-side stationary blocks (host-built, replicated to all cores)
    S = z64[:, None, :] + z64[None, :, :]                  # (m, m, q)
    zl = np.empty((34, M * M), np.float32)
    zl[0:16] = S.transpose(2, 0, 1).reshape(Q, M * M)
    zl[16:32] = (-0.25 * S * S).transpose(2, 0, 1).reshape(Q, M * M)
    zl[32] = 1.0
    sqz = (z64[:, None, :] - z64[None, :, :]) ** 2          # (m, m, q)
    s1 = 0.25 * (sqz @ al)                                  # (m, m)
    zl[33] = (-s1).reshape(M * M)

    zt = z64.T                                              # (q, m)
    zs1 = np.empty((33, M), np.float32)
    zs1[0:16] = zt
    zs1[16:32] = -0.5 * zt * zt
    zs1[32] = 1.0

    consts = np.tile(np.array([[2.0 * logvar, 4.0 * logvar, 0.0, 0.0]],
                              np.float32), (128, 1))
    alpha_in = np.tile(alpha.reshape(1, Q).astype(np.float32), (128, 1))

    in_maps = []
    for i in range(NCORES):
        sl = slice(i * NLOC, (i + 1) * NLOC)
        in_maps.append({
            "y": np.ascontiguousarray(y[sl], dtype=np.float32),
            "qmu": np.ascontiguousarray(q_mu[sl], dtype=np.float32),
            "qls": np.ascontiguousarray(q_log_sigma[sl], dtype=np.float32),
            "zl": zl,
            "zs1": zs1,
            "alpha": alpha_in,
            "consts": consts,
        })

    br = run_bass_kernel_spmd(nc, in_maps, list(range(NCORES)), trace=_trace)
    res = br.results

    psi2_part = np.zeros((128, 32), f8)
    A = np.zeros((M, D), f8)
    misc = np.zeros(8, f8)
    for r in res:
        psi2_part += r["out_psi2"].astype(f8)
        A += r["out_A"].astype(f8)
        misc += r["out_misc"].astype(f8).sum(axis=0)

    psi2 = psi2_part.T.reshape(M * M)[: M * M].reshape(M, M)
    lnsig = misc[0] + misc[1]
    ssq = misc[2] + misc[3]
    musq = misc[4] + misc[5]
    tr_yy = misc[6] + misc[7]

    kl_sum = -lnsig + 0.5 * (ssq + musq) - 0.5 * N * Q
    kl_term = kl_sum / (N * D)

    # small m x m algebra on host
    k_mm = var * np.exp(-0.5 * (sqz @ al))                  # (m, m)
    noise_var = _np_softplus(f8(noise_raw[0]))
    beta = 1.0 / noise_var
    psi0 = N * var

    cov1 = beta * psi2 + k_mm
    B = np.linalg.solve(cov1, A)
    tr_yWy = beta * tr_yy - np.sum(A * B)

    F = 0.5 * N * np.log(beta)
    F += 0.5 * np.linalg.slogdet(k_mm)[1]
    F -= 0.5 * N * np.log(np.pi)
    F -= 0.5 * np.linalg.slogdet(cov1)[1]
    F -= 0.5 * beta * psi0
    F += 0.5 * np.trace(np.linalg.solve(k_mm, psi2))
    F = (F * D - 0.5 * tr_yWy) / (N * D)

    out = F - kl_term
    result = np.asarray(out, dtype=np.float32)
    if _trace:
        return result, br
    return result


# revision 13
# speedup vs baseline: 1.5867x; 1.0041x over previous
"""Bayesian GPLVM collapsed-ELBO kernel for Trainium2 (8 NeuronCores).

Sharding: data-parallel over n (2048 rows -> 256 per core). Each core
computes its partial psi2 = sum_n exp(log_psi2_n) (m*m = 4096 entries),
partial A = psi1^T y (64x256), and partial row statistics (KL pieces,
sum y^2). Host sums the 8 partials and does the small m x m linear
algebra (Cholesky solves / slogdets) to produce the scalar ELBO.

Device layout per core (n_loc = 256, two 128-row chunks):
  - NPREP (98 x 256, q-major n-side): rows 0..15 = (q_mu*w1)^T,
    16..31 = w1^T, 32 = h1, 64..79 = (q_mu*w2)^T, 80..95 = w2^T,
    96 = g, 97 = ones (matmul operands need base partition in
    {0,32,64}, so the psi2 block sits at 64). Built n-major as a
    (128 x 98) tile per chunk, then PE-transposed.
  - psi1 exponent = NPREP[0:33,chunk]^T @ ZS1 (z-side, host-built),
    one matmul + Exp per chunk; A accumulates psi1^T y in PSUM.
  - psi2 exponent for each of 32 ij-chunks (128 ij-pairs each) =
    ZL[64:98, chunk]^T @ NPREP[64:98]; Exp with fused free-axis
    accumulation gives the local n-sum directly.
"""

import numpy as np

N, D, Q, M = 2048, 256, 16, 64
NCORES = 8
NLOC = N // NCORES          # 256
F32 = None                  # set lazily (mybir.dt.float32)

_compiled = None


def _build_bass():
    import concourse.bacc as bacc
    import concourse.bass as bass  # noqa: F401
    import concourse.mybir as mybir
    from concourse import masks
    from concourse.tile import TileContext

    f32 = mybir.dt.float32
    f32r = mybir.dt.float32r
    AF = mybir.ActivationFunctionType
    OP = mybir.AluOpType

    nc = bacc.Bacc("TRN2", target_bir_lowering=False)

    y_d = nc.declare_dram_parameter("y", [NLOC, D], f32r, isOutput=False)
    qmu_d = nc.declare_dram_parameter("qmu", [NLOC, Q], f32, isOutput=False)
    qls_d = nc.declare_dram_parameter("qls", [NLOC, Q], f32, isOutput=False)
    zl_d = nc.declare_dram_parameter("zl", [34, M * M], f32r, isOutput=False)
    zs1_d = nc.declare_dram_parameter("zs1", [33, M], f32r, isOutput=False)
    alpha_d = nc.declare_dram_parameter("alpha", [128, Q], f32, isOutput=False)
    consts_d = nc.declare_dram_parameter("consts", [128, 4], f32, isOutput=False)
    psi2_o = nc.declare_dram_parameter("out_psi2", [128, 32], f32, isOutput=True)
    a_o = nc.declare_dram_parameter("out_A", [M, D], f32, isOutput=True)
    misc_o = nc.declare_dram_parameter("out_misc", [128, 8], f32, isOutput=True)

    with TileContext(nc) as tc:
        with (
            tc.tile_pool(name="const", bufs=1) as cpool,
            tc.tile_pool(name="big", bufs=1) as bigpool,
            tc.tile_pool(name="work", bufs=3) as wpool,
            tc.tile_pool(name="scr", bufs=3) as spool,
            tc.tile_pool(name="psum", bufs=2, space="PSUM") as ppool,
            tc.tile_pool(name="psums", bufs=1, space="PSUM") as ppools,
            tc.tile_pool(name="psum1", bufs=1, space="PSUM") as ppool1,
        ):
            ident = cpool.tile([128, 128], f32)
            masks.make_identity(nc, ident[:])

            alpha_b = cpool.tile([128, Q], f32)
            nc.sync.dma_start(out=alpha_b[:, :], in_=alpha_d[:, :])

            consts_b = cpool.tile([128, 4], f32)
            nc.sync.dma_start(out=consts_b[:, :], in_=consts_d[:, :])

            zl_sb = bigpool.tile([98, M * M], f32r)
            zs1_sb = cpool.tile([33, M], f32r)
            nc.sync.dma_start(out=zs1_sb[:, :], in_=zs1_d[:, :])

            nprep = bigpool.tile([98, 2 * 128], f32r)
            stats = bigpool.tile([128, 32], f32)
            misc = bigpool.tile([128, 8], f32)
            apsum = ppool1.tile([M, D], f32)

            for c in range(2):
                r0, r1 = c * 128, (c + 1) * 128
                qm = wpool.tile([128, Q], f32, tag="qm")
                nc.sync.dma_start(out=qm[:, :], in_=qmu_d[r0:r1, :])
                qls = wpool.tile([128, Q], f32, tag="qls")
                nc.sync.dma_start(out=qls[:, :], in_=qls_d[r0:r1, :])
                yc = wpool.tile([128, D], f32r, tag="yc")
                nc.sync.dma_start(out=yc[:, :], in_=y_d[r0:r1, :])

                prepn = wpool.tile([128, 98], f32, tag="prepn")
                qsig = wpool.tile([128, Q], f32, tag="qsig")
                d1 = wpool.tile([128, Q], f32, tag="d1")
                d2 = wpool.tile([128, Q], f32, tag="d2")
                rcp = wpool.tile([128, Q], f32, tag="rcp")
                scr16 = spool.tile([128, Q], f32, tag="scr16")
                scrY = spool.tile([128, D], f32, tag="scrY")
                cols = wpool.tile([128, 8], f32, tag="cols")
                sum2c = cols[:, 0:1]
                s3x2c = cols[:, 1:2]
                rt1c = cols[:, 2:3]
                ac = cols[:, 3:4]
                t1c = cols[:, 4:5]
                t2c = cols[:, 5:6]

                # q_sigma = softplus(qls) = ln(1 + exp(qls))
                nc.scalar.activation(scr16[:, :], qls[:, :], AF.Exp)
                nc.scalar.activation(qsig[:, :], scr16[:, :], AF.Ln, bias=1.0)
                nc.vector.tensor_mul(d1[:, :], qsig[:, :], alpha_b[:, :])
                nc.vector.tensor_scalar_add(d1[:, :], d1[:, :], 1.0)
                # w1 = alpha / d1
                nc.vector.reciprocal(rcp[:, :], d1[:, :])
                nc.vector.tensor_mul(prepn[:, 16:32], rcp[:, :], alpha_b[:, :])
                # sum2 = sum_q log d1
                nc.scalar.activation(scr16[:, :], d1[:, :], AF.Ln, accum_out=sum2c)
                # d2 = 2*d1 - 1;  w2 = alpha / d2
                nc.vector.tensor_scalar(
                    out=d2[:, :], in0=d1[:, :], scalar1=2.0, scalar2=-1.0,
                    op0=OP.mult, op1=OP.add)
                nc.vector.reciprocal(rcp[:, :], d2[:, :])
                nc.vector.tensor_mul(prepn[:, 80:96], rcp[:, :], alpha_b[:, :])
                # 2*s3 = sum_q log d2
                nc.scalar.activation(scr16[:, :], d2[:, :], AF.Ln, accum_out=s3x2c)
                # q_mu * w1, q_mu * w2
                nc.vector.tensor_mul(prepn[:, 0:16], qm[:, :], prepn[:, 16:32])
                nc.vector.tensor_mul(prepn[:, 64:80], qm[:, :], prepn[:, 80:96])
                # rt1 = sum_q q_mu^2 w1 ; a = sum_q q_mu^2 w2
                nc.vector.tensor_mul(scr16[:, :], prepn[:, 0:16], qm[:, :])
                nc.vector.tensor_reduce(rt1c, scr16[:, :],
                                        axis=mybir.AxisListType.X, op=OP.add)
                nc.vector.tensor_mul(scr16[:, :], prepn[:, 64:80], qm[:, :])
                nc.vector.tensor_reduce(ac, scr16[:, :],
                                        axis=mybir.AxisListType.X, op=OP.add)
                # h1 = 2*logvar - 0.5*(rt1 + sum2)
                nc.vector.tensor_add(t1c, rt1c, sum2c)
                nc.vector.tensor_scalar(
                    out=prepn[:, 32:33], in0=t1c, scalar1=-0.5,
                    scalar2=consts_b[:, 0:1], op0=OP.mult, op1=OP.add)
                # g = 4*logvar - 0.5*(2*s3) - a
                nc.vector.tensor_scalar(
                    out=t2c, in0=s3x2c, scalar1=0.5, scalar2=ac,
                    op0=OP.mult, op1=OP.add)
                nc.vector.tensor_scalar(
                    out=prepn[:, 96:97], in0=t2c, scalar1=-1.0,
                    scalar2=consts_b[:, 1:2], op0=OP.mult, op1=OP.add)
                nc.vector.memset(prepn[:, 97:98], 1.0)
                nc.vector.memset(prepn[:, 33:64], 0.0)

                # KL / trace statistics (squares on DVE, Ln stays on ACT)
                nc.scalar.activation(scr16[:, :], qsig[:, :], AF.Ln,
                                     accum_out=misc[:, 0 + c:1 + c])
                nc.vector.tensor_mul(scr16[:, :], qsig[:, :], qsig[:, :])
                nc.vector.tensor_reduce(misc[:, 2 + c:3 + c], scr16[:, :],
                                        axis=mybir.AxisListType.X, op=OP.add)
                nc.vector.tensor_mul(scr16[:, :], qm[:, :], qm[:, :])
                nc.vector.tensor_reduce(misc[:, 4 + c:5 + c], scr16[:, :],
                                        axis=mybir.AxisListType.X, op=OP.add)
                nc.vector.tensor_mul(scrY[:, :], yc[:, :].bitcast(f32), yc[:, :].bitcast(f32))
                nc.vector.tensor_reduce(misc[:, 6 + c:7 + c], scrY[:, :],
                                        axis=mybir.AxisListType.X, op=OP.add)

                # transpose prep (128 x 67) -> NPREP[:, chunk]
                ptp = ppools.tile([98, 128], f32, tag="ptp")
                nc.tensor.transpose(ptp[:, :], prepn[:, :], ident[:, :])
                nc.vector.tensor_copy(nprep[:, r0:r1], ptp[:, :])

                # psi1 chunk: exponent (128 n x 64 m) then exp
                e1 = ppools.tile([128, M], f32, tag="e1")
                nc.tensor.matmul(e1[:, :],
                                 lhsT=nprep[0:33, r0:r1],
                                 rhs=zs1_sb[:, :],
                                 start=True, stop=True)
                psi1c = wpool.tile([128, M], f32r, tag="psi1c")
                nc.scalar.activation(psi1c[:, :], e1[:, :], AF.Exp)
                # A += psi1_c^T @ y_c
                nc.tensor.matmul(apsum[:, :], lhsT=psi1c[:, :],
                                 rhs=yc[:, :],
                                 start=(c == 0), stop=(c == 1))

            # zl is big (544KB): issue on the gpsimd queue, after the small
            # sync-queue DMAs, so they don't queue behind it
            nc.gpsimd.dma_start(out=zl_sb[64:98, :], in_=zl_d[:, :])

            # psi2: 32 ij-chunks of 128 pairs, 4 chunks per PSUM tile
            for t in range(8):
                p2 = ppool.tile([128, 4 * NLOC], f32, tag="p2")
                for j in range(4):
                    ch = 4 * t + j
                    nc.tensor.matmul(
                        p2[:, j * NLOC:(j + 1) * NLOC],
                        lhsT=zl_sb[64:98, ch * 128:(ch + 1) * 128],
                        rhs=nprep[64:98, :],
                        start=True, stop=True)
                scr = spool.tile([128, 4 * NLOC], f32, tag="p2scr")
                nc.scalar.activation(scr[:, :], p2[:, :], AF.Exp)
                nc.vector.tensor_reduce(
                    stats[:, 4 * t:4 * t + 4],
                    scr[:, :].rearrange("p (a b) -> p a b", b=NLOC),
                    axis=mybir.AxisListType.X, op=OP.add)

            a_sb = bigpool.tile([M, D], f32)
            nc.vector.tensor_copy(a_sb[:, :], apsum[:, :])
            nc.sync.dma_start(out=psi2_o[:, :], in_=stats[:, :])
            nc.sync.dma_start(out=a_o[:, :], in_=a_sb[:, :])
            nc.sync.dma_start(out=misc_o[:, :], in_=misc[:, :])

    nc.compile()
    return nc


def _get_compiled():
    global _compiled
    if _compiled is None:
        _compiled = _build_bass()
    return _compiled


def _np_softplus(x):
    return np.logaddexp(x, 0.0)


def kernel(y, q_mu, q_log_sigma, z, noise_raw, alpha, variance, _trace=False):
    from concourse.bass_utils import run_bass_kernel_spmd

    nc = _get_compiled()

    f8 = np.float64
    z64 = z.astype(f8)
    al = alpha.astype(f8)
    var = f8(variance[0])
    logvar = np.log(var)

    # z-side stationary blocks (host-built, replicated to all cores)
    S = z64[:, None, :] + z64[None, :, :]                  # (m, m, q)
    zl = np.empty((34, M * M), np.float32)
    zl[0:16] = S.transpose(2, 0, 1).reshape(Q, M * M)
    zl[16:32] = (-0.25 * S * S).transpose(2, 0, 1).reshape(Q, M * M)
    zl[32] = 1.0
    sqz = (z64[:, None, :] - z64[None, :, :]) ** 2          # (m, m, q)
    s1 = 0.25 * (sqz @ al)                                  # (m, m)
    zl[33] = (-s1).reshape(M * M)

    zt = z64.T                                              # (q, m)
    zs1 = np.empty((33, M), np.float32)
    zs1[0:16] = zt
    zs1[16:32] = -0.5 * zt * zt
    zs1[32] = 1.0

    consts = np.tile(np.array([[2.0 * logvar, 4.0 * logvar, 0.0, 0.0]],
                              np.float32), (128, 1))
    alpha_in = np.tile(alpha.reshape(1, Q).astype(np.float32), (128, 1))

    in_maps = []
    for i in range(NCORES):
        sl = slice(i * NLOC, (i + 1) * NLOC)
        in_maps.append({
            "y": np.ascontiguousarray(y[sl], dtype=np.float32),
            "qmu": np.ascontiguousarray(q_mu[sl], dtype=np.float32),
            "qls": np.ascontiguousarray(q_log_sigma[sl], dtype=np.float32),
            "zl": zl,
            "zs1": zs1,
            "alpha": alpha_in,
            "consts": consts,
        })

    br = run_bass_kernel_spmd(nc, in_maps, list(range(NCORES)), trace=_trace)
    res = br.results

    psi2_part = np.zeros((128, 32), f8)
    A = np.zeros((M, D), f8)
    misc = np.zeros(8, f8)
    for r in res:
        psi2_part += r["out_psi2"].astype(f8)
        A += r["out_A"].astype(f8)
        misc += r["out_misc"].astype(f8).sum(axis=0)

    psi2 = psi2_part.T.reshape(M * M)[: M * M].reshape(M, M)
    lnsig = misc[0] + misc[1]
    ssq = misc[2] + misc[3]
    musq = misc[4] + misc[5]
    tr_yy = misc[6] + misc[7]

    kl_sum = -lnsig + 0.5 * (ssq + musq) - 0.5 * N * Q
    kl_term = kl_sum / (N * D)

    # small m x m algebra on host
    k_mm = var * np.exp(-0.5 * (sqz @ al))                  # (m, m)
    noise_var = _np_softplus(f8(noise_raw[0]))
    beta = 1.0 / noise_var
    psi0 = N * var

    cov1 = beta * psi2 + k_mm
    B = np.linalg.solve(cov1, A)
    tr_yWy = beta * tr_yy - np.sum(A * B)

    F = 0.5 * N * np.log(beta)
    F += 0.5 * np.linalg.slogdet(k_mm)[1]
    F -= 0.5 * N * np.log(np.pi)
    F -= 0.5 * np.linalg.slogdet(cov1)[1]
    F -= 0.5 * beta * psi0
    F += 0.5 * np.trace(np.linalg.solve(k_mm, psi2))
    F = (F * D - 0.5 * tr_yWy) / (N * D)

    out = F - kl_term
    result = np.asarray(out, dtype=np.float32)
    if _trace:
        return result, br
    return result


# revision 14
# speedup vs baseline: 1.9759x; 1.2453x over previous
"""Bayesian GPLVM collapsed-ELBO kernel for Trainium2 (8 NeuronCores).

Sharding: data-parallel over n (2048 rows -> 256 per core). Each core
computes its partial psi2 = sum_n exp(log_psi2_n) (m*m = 4096 entries),
partial A = psi1^T y (64x256), and partial row statistics (KL pieces,
sum y^2). Host sums the 8 partials and does the small m x m linear
algebra (Cholesky solves / slogdets) to produce the scalar ELBO.

Device layout per core (n_loc = 256, two 128-row chunks):
  - NPREP (98 x 256, q-major n-side): rows 0..15 = (q_mu*w1)^T,
    16..31 = w1^T, 32 = h1, 64..79 = (q_mu*w2)^T, 80..95 = w2^T,
    96 = g, 97 = ones (matmul operands need base partition in
    {0,32,64}, so the psi2 block sits at 64). Built n-major as a
    (128 x 98) tile per chunk, then PE-transposed.
  - psi1 exponent = NPREP[0:33,chunk]^T @ ZS1 (z-side, host-built),
    one matmul + Exp per chunk; A accumulates psi1^T y in PSUM.
  - psi2 exponent for each of 32 ij-chunks (128 ij-pairs each) =
    ZL[64:98, chunk]^T @ NPREP[64:98]; Exp with fused free-axis
    accumulation gives the local n-sum directly.
"""

import numpy as np

N, D, Q, M = 2048, 256, 16, 64
NCORES = 8
NLOC = N // NCORES          # 256
F32 = None                  # set lazily (mybir.dt.float32)

_compiled = None


def _build_bass():
    import concourse.bacc as bacc
    import concourse.bass as bass  # noqa: F401
    import concourse.mybir as mybir
    from concourse import masks
    from concourse.tile import TileContext

    f32 = mybir.dt.float32
    f32r = mybir.dt.float32r
    AF = mybir.ActivationFunctionType
    OP = mybir.AluOpType

    nc = bacc.Bacc("TRN2", target_bir_lowering=False)

    y_d = nc.declare_dram_parameter("y", [NLOC, D], f32r, isOutput=False)
    qmu_d = nc.declare_dram_parameter("qmu", [NLOC, Q], f32, isOutput=False)
    qls_d = nc.declare_dram_parameter("qls", [NLOC, Q], f32, isOutput=False)
    zl_d = nc.declare_dram_parameter("zl", [34, 17 * 128], f32r, isOutput=False)
    zs1_d = nc.declare_dram_parameter("zs1", [33, M], f32r, isOutput=False)
    alpha_d = nc.declare_dram_parameter("alpha", [128, Q], f32, isOutput=False)
    consts_d = nc.declare_dram_parameter("consts", [128, 4], f32, isOutput=False)
    psi2_o = nc.declare_dram_parameter("out_psi2", [128, 17], f32, isOutput=True)
    a_o = nc.declare_dram_parameter("out_A", [M, D], f32, isOutput=True)
    misc_o = nc.declare_dram_parameter("out_misc", [128, 8], f32, isOutput=True)

    with TileContext(nc) as tc:
        with (
            tc.tile_pool(name="const", bufs=1) as cpool,
            tc.tile_pool(name="big", bufs=1) as bigpool,
            tc.tile_pool(name="work", bufs=3) as wpool,
            tc.tile_pool(name="scr", bufs=3) as spool,
            tc.tile_pool(name="psum", bufs=2, space="PSUM") as ppool,
            tc.tile_pool(name="psums", bufs=1, space="PSUM") as ppools,
            tc.tile_pool(name="psum1", bufs=1, space="PSUM") as ppool1,
        ):
            ident = cpool.tile([128, 128], f32)
            masks.make_identity(nc, ident[:])

            alpha_b = cpool.tile([128, Q], f32)
            nc.sync.dma_start(out=alpha_b[:, :], in_=alpha_d[:, :])

            consts_b = cpool.tile([128, 4], f32)
            nc.sync.dma_start(out=consts_b[:, :], in_=consts_d[:, :])

            zl_sb = bigpool.tile([98, 17 * 128], f32r)
            zs1_sb = cpool.tile([33, M], f32r)
            nc.sync.dma_start(out=zs1_sb[:, :], in_=zs1_d[:, :])

            nprep = bigpool.tile([98, 2 * 128], f32r)
            stats = bigpool.tile([128, 17], f32)
            misc = bigpool.tile([128, 8], f32)
            apsum = ppool1.tile([M, D], f32)

            for c in range(2):
                r0, r1 = c * 128, (c + 1) * 128
                qm = wpool.tile([128, Q], f32, tag="qm")
                nc.sync.dma_start(out=qm[:, :], in_=qmu_d[r0:r1, :])
                qls = wpool.tile([128, Q], f32, tag="qls")
                nc.sync.dma_start(out=qls[:, :], in_=qls_d[r0:r1, :])
                yc = wpool.tile([128, D], f32r, tag="yc")
                nc.sync.dma_start(out=yc[:, :], in_=y_d[r0:r1, :])

                prepn = wpool.tile([128, 98], f32, tag="prepn")
                qsig = wpool.tile([128, Q], f32, tag="qsig")
                d1 = wpool.tile([128, Q], f32, tag="d1")
                d2 = wpool.tile([128, Q], f32, tag="d2")
                rcp = wpool.tile([128, Q], f32, tag="rcp")
                scr16 = spool.tile([128, Q], f32, tag="scr16")
                scrY = spool.tile([128, D], f32, tag="scrY")
                cols = wpool.tile([128, 8], f32, tag="cols")
                sum2c = cols[:, 0:1]
                s3x2c = cols[:, 1:2]
                rt1c = cols[:, 2:3]
                ac = cols[:, 3:4]
                t1c = cols[:, 4:5]
                t2c = cols[:, 5:6]

                # q_sigma = softplus(qls) = ln(1 + exp(qls))
                nc.scalar.activation(scr16[:, :], qls[:, :], AF.Exp)
                nc.scalar.activation(qsig[:, :], scr16[:, :], AF.Ln, bias=1.0)
                nc.vector.tensor_mul(d1[:, :], qsig[:, :], alpha_b[:, :])
                nc.vector.tensor_scalar_add(d1[:, :], d1[:, :], 1.0)
                # w1 = alpha / d1
                nc.vector.reciprocal(rcp[:, :], d1[:, :])
                nc.vector.tensor_mul(prepn[:, 16:32], rcp[:, :], alpha_b[:, :])
                # sum2 = sum_q log d1
                nc.scalar.activation(scr16[:, :], d1[:, :], AF.Ln, accum_out=sum2c)
                # d2 = 2*d1 - 1;  w2 = alpha / d2
                nc.vector.tensor_scalar(
                    out=d2[:, :], in0=d1[:, :], scalar1=2.0, scalar2=-1.0,
                    op0=OP.mult, op1=OP.add)
                nc.vector.reciprocal(rcp[:, :], d2[:, :])
                nc.vector.tensor_mul(prepn[:, 80:96], rcp[:, :], alpha_b[:, :])
                # 2*s3 = sum_q log d2
                nc.scalar.activation(scr16[:, :], d2[:, :], AF.Ln, accum_out=s3x2c)
                # q_mu * w1, q_mu * w2
                nc.vector.tensor_mul(prepn[:, 0:16], qm[:, :], prepn[:, 16:32])
                nc.vector.tensor_mul(prepn[:, 64:80], qm[:, :], prepn[:, 80:96])
                # rt1 = sum_q q_mu^2 w1 ; a = sum_q q_mu^2 w2
                nc.vector.tensor_mul(scr16[:, :], prepn[:, 0:16], qm[:, :])
                nc.vector.tensor_reduce(rt1c, scr16[:, :],
                                        axis=mybir.AxisListType.X, op=OP.add)
                nc.vector.tensor_mul(scr16[:, :], prepn[:, 64:80], qm[:, :])
                nc.vector.tensor_reduce(ac, scr16[:, :],
                                        axis=mybir.AxisListType.X, op=OP.add)
                # h1 = 2*logvar - 0.5*(rt1 + sum2)
                nc.vector.tensor_add(t1c, rt1c, sum2c)
                nc.vector.tensor_scalar(
                    out=prepn[:, 32:33], in0=t1c, scalar1=-0.5,
                    scalar2=consts_b[:, 0:1], op0=OP.mult, op1=OP.add)
                # g = 4*logvar - 0.5*(2*s3) - a
                nc.vector.tensor_scalar(
                    out=t2c, in0=s3x2c, scalar1=0.5, scalar2=ac,
                    op0=OP.mult, op1=OP.add)
                nc.vector.tensor_scalar(
                    out=prepn[:, 96:97], in0=t2c, scalar1=-1.0,
                    scalar2=consts_b[:, 1:2], op0=OP.mult, op1=OP.add)
                nc.vector.memset(prepn[:, 97:98], 1.0)
                nc.vector.memset(prepn[:, 33:64], 0.0)

                # KL / trace statistics (squares on DVE, Ln stays on ACT)
                nc.scalar.activation(scr16[:, :], qsig[:, :], AF.Ln,
                                     accum_out=misc[:, 0 + c:1 + c])
                nc.vector.tensor_mul(scr16[:, :], qsig[:, :], qsig[:, :])
                nc.vector.tensor_reduce(misc[:, 2 + c:3 + c], scr16[:, :],
                                        axis=mybir.AxisListType.X, op=OP.add)
                nc.vector.tensor_mul(scr16[:, :], qm[:, :], qm[:, :])
                nc.vector.tensor_reduce(misc[:, 4 + c:5 + c], scr16[:, :],
                                        axis=mybir.AxisListType.X, op=OP.add)
                nc.vector.tensor_mul(scrY[:, :], yc[:, :].bitcast(f32), yc[:, :].bitcast(f32))
                nc.vector.tensor_reduce(misc[:, 6 + c:7 + c], scrY[:, :],
                                        axis=mybir.AxisListType.X, op=OP.add)

                # transpose prep (128 x 67) -> NPREP[:, chunk]
                ptp = ppools.tile([98, 128], f32, tag="ptp")
                nc.tensor.transpose(ptp[:, :], prepn[:, :], ident[:, :])
                nc.vector.tensor_copy(nprep[:, r0:r1], ptp[:, :])

                # psi1 chunk: exponent (128 n x 64 m) then exp
                e1 = ppools.tile([128, M], f32, tag="e1")
                nc.tensor.matmul(e1[:, :],
                                 lhsT=nprep[0:33, r0:r1],
                                 rhs=zs1_sb[:, :],
                                 start=True, stop=True)
                psi1c = wpool.tile([128, M], f32r, tag="psi1c")
                nc.scalar.activation(psi1c[:, :], e1[:, :], AF.Exp)
                # A += psi1_c^T @ y_c
                nc.tensor.matmul(apsum[:, :], lhsT=psi1c[:, :],
                                 rhs=yc[:, :],
                                 start=(c == 0), stop=(c == 1))

            # zl is big (544KB): issue on the gpsimd queue, after the small
            # sync-queue DMAs, so they don't queue behind it
            nc.gpsimd.dma_start(out=zl_sb[64:98, :], in_=zl_d[:, :])

            # psi2 is symmetric: only the 2080 upper-triangle ij-pairs
            # (17 chunks of 128, last 96 slots are padding), 4 chunks per
            # PSUM tile
            for t in range(5):
                nch = min(4, 17 - 4 * t)
                p2 = ppool.tile([128, 4 * NLOC], f32, tag="p2")
                for j in range(nch):
                    ch = 4 * t + j
                    nc.tensor.matmul(
                        p2[:, j * NLOC:(j + 1) * NLOC],
                        lhsT=zl_sb[64:98, ch * 128:(ch + 1) * 128],
                        rhs=nprep[64:98, :],
                        start=True, stop=True)
                scr = spool.tile([128, 4 * NLOC], f32, tag="p2scr")
                w = nch * NLOC
                nc.scalar.activation(scr[:, :w], p2[:, :w], AF.Exp)
                nc.vector.tensor_reduce(
                    stats[:, 4 * t:4 * t + nch],
                    scr[:, :w].rearrange("p (a b) -> p a b", b=NLOC),
                    axis=mybir.AxisListType.X, op=OP.add)

            a_sb = bigpool.tile([M, D], f32)
            nc.vector.tensor_copy(a_sb[:, :], apsum[:, :])
            nc.sync.dma_start(out=psi2_o[:, :], in_=stats[:, :])
            nc.sync.dma_start(out=a_o[:, :], in_=a_sb[:, :])
            nc.sync.dma_start(out=misc_o[:, :], in_=misc[:, :])

    nc.compile()
    return nc


def _get_compiled():
    global _compiled
    if _compiled is None:
        _compiled = _build_bass()
    return _compiled


def _np_softplus(x):
    return np.logaddexp(x, 0.0)


def kernel(y, q_mu, q_log_sigma, z, noise_raw, alpha, variance, _trace=False):
    from concourse.bass_utils import run_bass_kernel_spmd

    nc = _get_compiled()

    f8 = np.float64
    z64 = z.astype(f8)
    al = alpha.astype(f8)
    var = f8(variance[0])
    logvar = np.log(var)

    # z-side stationary blocks (host-built, replicated to all cores).
    # psi2 is symmetric in (i, j): ship only the 2080 upper-tri pairs.
    iu, ju = np.triu_indices(M)                             # (2080,)
    npairs = iu.shape[0]
    Su = z64[iu] + z64[ju]                                  # (2080, q)
    sqz = (z64[:, None, :] - z64[None, :, :]) ** 2          # (m, m, q)
    s1 = 0.25 * (sqz @ al)                                  # (m, m)
    zl = np.zeros((34, 17 * 128), np.float32)
    zl[0:16, :npairs] = Su.T
    zl[16:32, :npairs] = (-0.25 * Su * Su).T
    zl[32, :npairs] = 1.0
    zl[33, :npairs] = -s1[iu, ju]

    zt = z64.T                                              # (q, m)
    zs1 = np.empty((33, M), np.float32)
    zs1[0:16] = zt
    zs1[16:32] = -0.5 * zt * zt
    zs1[32] = 1.0

    consts = np.tile(np.array([[2.0 * logvar, 4.0 * logvar, 0.0, 0.0]],
                              np.float32), (128, 1))
    alpha_in = np.tile(alpha.reshape(1, Q).astype(np.float32), (128, 1))

    in_maps = []
    for i in range(NCORES):
        sl = slice(i * NLOC, (i + 1) * NLOC)
        in_maps.append({
            "y": np.ascontiguousarray(y[sl], dtype=np.float32),
            "qmu": np.ascontiguousarray(q_mu[sl], dtype=np.float32),
            "qls": np.ascontiguousarray(q_log_sigma[sl], dtype=np.float32),
            "zl": zl,
            "zs1": zs1,
            "alpha": alpha_in,
            "consts": consts,
        })

    br = run_bass_kernel_spmd(nc, in_maps, list(range(NCORES)), trace=_trace)
    res = br.results

    psi2_part = np.zeros((128, 17), f8)
    A = np.zeros((M, D), f8)
    misc = np.zeros(8, f8)
    for r in res:
        psi2_part += r["out_psi2"].astype(f8)
        A += r["out_A"].astype(f8)
        misc += r["out_misc"].astype(f8).sum(axis=0)

    flat = psi2_part.T.reshape(17 * 128)
    psi2 = np.empty((M, M), f8)
    psi2[iu, ju] = flat[:npairs]
    psi2[ju, iu] = flat[:npairs]
    lnsig = misc[0] + misc[1]
    ssq = misc[2] + misc[3]
    musq = misc[4] + misc[5]
    tr_yy = misc[6] + misc[7]

    kl_sum = -lnsig + 0.5 * (ssq + musq) - 0.5 * N * Q
    kl_term = kl_sum / (N * D)

    # small m x m algebra on host
    k_mm = var * np.exp(-0.5 * (sqz @ al))                  # (m, m)
    noise_var = _np_softplus(f8(noise_raw[0]))
    beta = 1.0 / noise_var
    psi0 = N * var

    cov1 = beta * psi2 + k_mm
    B = np.linalg.solve(cov1, A)
    tr_yWy = beta * tr_yy - np.sum(A * B)

    F = 0.5 * N * np.log(beta)
    F += 0.5 * np.linalg.slogdet(k_mm)[1]
    F -= 0.5 * N * np.log(np.pi)
    F -= 0.5 * np.linalg.slogdet(cov1)[1]
    F -= 0.5 * beta * psi0
    F += 0.5 * np.trace(np.linalg.solve(k_mm, psi2))
    F = (F * D - 0.5 * tr_yWy) / (N * D)

    out = F - kl_term
    result = np.asarray(out, dtype=np.float32)
    if _trace:
        return result, br
    return result


# revision 15
# speedup vs baseline: 1.9779x; 1.0010x over previous
"""Bayesian GPLVM collapsed-ELBO kernel for Trainium2 (8 NeuronCores).

Sharding: data-parallel over n (2048 rows -> 256 per core). Each core
computes its partial psi2 = sum_n exp(log_psi2_n) (m*m = 4096 entries),
partial A = psi1^T y (64x256), and partial row statistics (KL pieces,
sum y^2). Host sums the 8 partials and does the small m x m linear
algebra (Cholesky solves / slogdets) to produce the scalar ELBO.

Device layout per core (n_loc = 256, two 128-row chunks):
  - NPREP (98 x 256, q-major n-side): rows 0..15 = (q_mu*w1)^T,
    16..31 = w1^T, 32 = h1, 64..79 = (q_mu*w2)^T, 80..95 = w2^T,
    96 = g, 97 = ones (matmul operands need base partition in
    {0,32,64}, so the psi2 block sits at 64). Built n-major as a
    (128 x 98) tile per chunk, then PE-transposed.
  - psi1 exponent = NPREP[0:33,chunk]^T @ ZS1 (z-side, host-built),
    one matmul + Exp per chunk; A accumulates psi1^T y in PSUM.
  - psi2 exponent for each of 32 ij-chunks (128 ij-pairs each) =
    ZL[64:98, chunk]^T @ NPREP[64:98]; Exp with fused free-axis
    accumulation gives the local n-sum directly.
"""

import numpy as np

N, D, Q, M = 2048, 256, 16, 64
NCORES = 8
NLOC = N // NCORES          # 256
F32 = None                  # set lazily (mybir.dt.float32)

_compiled = None


def _build_bass():
    import concourse.bacc as bacc
    import concourse.bass as bass  # noqa: F401
    import concourse.mybir as mybir
    from concourse import masks
    from concourse.tile import TileContext

    f32 = mybir.dt.float32
    f32r = mybir.dt.float32r
    AF = mybir.ActivationFunctionType
    OP = mybir.AluOpType

    nc = bacc.Bacc("TRN2", target_bir_lowering=False, num_swdge_queues=2)

    y_d = nc.declare_dram_parameter("y", [NLOC, D], f32r, isOutput=False)
    qmu_d = nc.declare_dram_parameter("qmu", [NLOC, Q], f32, isOutput=False)
    qls_d = nc.declare_dram_parameter("qls", [NLOC, Q], f32, isOutput=False)
    zl_d = nc.declare_dram_parameter("zl", [34, 17 * 128], f32r, isOutput=False)
    zs1_d = nc.declare_dram_parameter("zs1", [33, M], f32r, isOutput=False)
    alpha_d = nc.declare_dram_parameter("alpha", [128, Q], f32, isOutput=False)
    consts_d = nc.declare_dram_parameter("consts", [128, 4], f32, isOutput=False)
    psi2_o = nc.declare_dram_parameter("out_psi2", [128, 17], f32, isOutput=True)
    a_o = nc.declare_dram_parameter("out_A", [M, D], f32, isOutput=True)
    misc_o = nc.declare_dram_parameter("out_misc", [128, 8], f32, isOutput=True)

    with TileContext(nc) as tc:
        with (
            tc.tile_pool(name="const", bufs=1) as cpool,
            tc.tile_pool(name="big", bufs=1) as bigpool,
            tc.tile_pool(name="work", bufs=3) as wpool,
            tc.tile_pool(name="scr", bufs=3) as spool,
            tc.tile_pool(name="psum", bufs=2, space="PSUM") as ppool,
            tc.tile_pool(name="psums", bufs=1, space="PSUM") as ppools,
            tc.tile_pool(name="psum1", bufs=1, space="PSUM") as ppool1,
        ):
            ident = cpool.tile([128, 128], f32)
            masks.make_identity(nc, ident[:])

            alpha_b = cpool.tile([128, Q], f32)
            nc.sync.dma_start(out=alpha_b[:, :], in_=alpha_d[:, :])

            consts_b = cpool.tile([128, 4], f32)
            nc.sync.dma_start(out=consts_b[:, :], in_=consts_d[:, :])

            zl_sb = bigpool.tile([98, 17 * 128], f32r)
            zs1_sb = cpool.tile([33, M], f32r)
            nc.sync.dma_start(out=zs1_sb[:, :], in_=zs1_d[:, :])

            nprep = bigpool.tile([98, 2 * 128], f32r)
            stats = bigpool.tile([128, 17], f32)
            misc = bigpool.tile([128, 8], f32)
            apsum = ppool1.tile([M, D], f32)

            for c in range(2):
                r0, r1 = c * 128, (c + 1) * 128
                qm = wpool.tile([128, Q], f32, tag="qm")
                nc.sync.dma_start(out=qm[:, :], in_=qmu_d[r0:r1, :])
                qls = wpool.tile([128, Q], f32, tag="qls")
                nc.sync.dma_start(out=qls[:, :], in_=qls_d[r0:r1, :])
                yc = wpool.tile([128, D], f32r, tag="yc")
                nc.sync.dma_start(out=yc[:, :], in_=y_d[r0:r1, :])

                prepn = wpool.tile([128, 98], f32, tag="prepn")
                qsig = wpool.tile([128, Q], f32, tag="qsig")
                d1 = wpool.tile([128, Q], f32, tag="d1")
                d2 = wpool.tile([128, Q], f32, tag="d2")
                rcp = wpool.tile([128, Q], f32, tag="rcp")
                scr16 = spool.tile([128, Q], f32, tag="scr16")
                scrY = spool.tile([128, D], f32, tag="scrY")
                cols = wpool.tile([128, 8], f32, tag="cols")
                sum2c = cols[:, 0:1]
                s3x2c = cols[:, 1:2]
                rt1c = cols[:, 2:3]
                ac = cols[:, 3:4]
                t1c = cols[:, 4:5]
                t2c = cols[:, 5:6]

                # q_sigma = softplus(qls) = ln(1 + exp(qls))
                nc.scalar.activation(scr16[:, :], qls[:, :], AF.Exp)
                nc.scalar.activation(qsig[:, :], scr16[:, :], AF.Ln, bias=1.0)
                nc.vector.tensor_mul(d1[:, :], qsig[:, :], alpha_b[:, :])
                nc.vector.tensor_scalar_add(d1[:, :], d1[:, :], 1.0)
                # w1 = alpha / d1
                nc.vector.reciprocal(rcp[:, :], d1[:, :])
                nc.vector.tensor_mul(prepn[:, 16:32], rcp[:, :], alpha_b[:, :])
                # sum2 = sum_q log d1
                nc.scalar.activation(scr16[:, :], d1[:, :], AF.Ln, accum_out=sum2c)
                # d2 = 2*d1 - 1;  w2 = alpha / d2
                nc.vector.tensor_scalar(
                    out=d2[:, :], in0=d1[:, :], scalar1=2.0, scalar2=-1.0,
                    op0=OP.mult, op1=OP.add)
                nc.vector.reciprocal(rcp[:, :], d2[:, :])
                nc.vector.tensor_mul(prepn[:, 80:96], rcp[:, :], alpha_b[:, :])
                # 2*s3 = sum_q log d2
                nc.scalar.activation(scr16[:, :], d2[:, :], AF.Ln, accum_out=s3x2c)
                # q_mu * w1, q_mu * w2
                nc.vector.tensor_mul(prepn[:, 0:16], qm[:, :], prepn[:, 16:32])
                nc.vector.tensor_mul(prepn[:, 64:80], qm[:, :], prepn[:, 80:96])
                # rt1 = sum_q q_mu^2 w1 ; a = sum_q q_mu^2 w2
                nc.vector.tensor_mul(scr16[:, :], prepn[:, 0:16], qm[:, :])
                nc.vector.tensor_reduce(rt1c, scr16[:, :],
                                        axis=mybir.AxisListType.X, op=OP.add)
                nc.vector.tensor_mul(scr16[:, :], prepn[:, 64:80], qm[:, :])
                nc.vector.tensor_reduce(ac, scr16[:, :],
                                        axis=mybir.AxisListType.X, op=OP.add)
                # h1 = 2*logvar - 0.5*(rt1 + sum2)
                nc.vector.tensor_add(t1c, rt1c, sum2c)
                nc.vector.tensor_scalar(
                    out=prepn[:, 32:33], in0=t1c, scalar1=-0.5,
                    scalar2=consts_b[:, 0:1], op0=OP.mult, op1=OP.add)
                # g = 4*logvar - 0.5*(2*s3) - a
                nc.vector.tensor_scalar(
                    out=t2c, in0=s3x2c, scalar1=0.5, scalar2=ac,
                    op0=OP.mult, op1=OP.add)
                nc.vector.tensor_scalar(
                    out=prepn[:, 96:97], in0=t2c, scalar1=-1.0,
                    scalar2=consts_b[:, 1:2], op0=OP.mult, op1=OP.add)
                nc.vector.memset(prepn[:, 97:98], 1.0)
                nc.vector.memset(prepn[:, 33:64], 0.0)

                # KL / trace statistics (squares on DVE, Ln stays on ACT)
                nc.scalar.activation(scr16[:, :], qsig[:, :], AF.Ln,
                                     accum_out=misc[:, 0 + c:1 + c])
                nc.vector.tensor_mul(scr16[:, :], qsig[:, :], qsig[:, :])
                nc.vector.tensor_reduce(misc[:, 2 + c:3 + c], scr16[:, :],
                                        axis=mybir.AxisListType.X, op=OP.add)
                nc.vector.tensor_mul(scr16[:, :], qm[:, :], qm[:, :])
                nc.vector.tensor_reduce(misc[:, 4 + c:5 + c], scr16[:, :],
                                        axis=mybir.AxisListType.X, op=OP.add)
                nc.vector.tensor_mul(scrY[:, :], yc[:, :].bitcast(f32), yc[:, :].bitcast(f32))
                nc.vector.tensor_reduce(misc[:, 6 + c:7 + c], scrY[:, :],
                                        axis=mybir.AxisListType.X, op=OP.add)

                # transpose prep (128 x 67) -> NPREP[:, chunk]
                ptp = ppools.tile([98, 128], f32, tag="ptp")
                nc.tensor.transpose(ptp[:, :], prepn[:, :], ident[:, :])
                nc.vector.tensor_copy(nprep[:, r0:r1], ptp[:, :])

                # psi1 chunk: exponent (128 n x 64 m) then exp
                e1 = ppools.tile([128, M], f32, tag="e1")
                nc.tensor.matmul(e1[:, :],
                                 lhsT=nprep[0:33, r0:r1],
                                 rhs=zs1_sb[:, :],
                                 start=True, stop=True)
                psi1c = wpool.tile([128, M], f32r, tag="psi1c")
                nc.scalar.activation(psi1c[:, :], e1[:, :], AF.Exp)
                # A += psi1_c^T @ y_c
                nc.tensor.matmul(apsum[:, :], lhsT=psi1c[:, :],
                                 rhs=yc[:, :],
                                 start=(c == 0), stop=(c == 1))

            # zl is big (544KB): issue on the gpsimd queue, after the small
            # sync-queue DMAs, so they don't queue behind it
            nc.gpsimd.dma_start(out=zl_sb[64:98, :], in_=zl_d[:, :])

            # psi2 is symmetric: only the 2080 upper-triangle ij-pairs
            # (17 chunks of 128, last 96 slots are padding), 4 chunks per
            # PSUM tile
            for t in range(5):
                nch = min(4, 17 - 4 * t)
                p2 = ppool.tile([128, 4 * NLOC], f32, tag="p2")
                for j in range(nch):
                    ch = 4 * t + j
                    nc.tensor.matmul(
                        p2[:, j * NLOC:(j + 1) * NLOC],
                        lhsT=zl_sb[64:98, ch * 128:(ch + 1) * 128],
                        rhs=nprep[64:98, :],
                        start=True, stop=True)
                scr = spool.tile([128, 4 * NLOC], f32, tag="p2scr")
                w = nch * NLOC
                nc.scalar.activation(scr[:, :w], p2[:, :w], AF.Exp)
                nc.vector.tensor_reduce(
                    stats[:, 4 * t:4 * t + nch],
                    scr[:, :w].rearrange("p (a b) -> p a b", b=NLOC),
                    axis=mybir.AxisListType.X, op=OP.add)

            a_sb = bigpool.tile([M, D], f32)
            nc.vector.tensor_copy(a_sb[:, :], apsum[:, :])
            nc.sync.dma_start(out=psi2_o[:, :], in_=stats[:, :])
            nc.sync.dma_start(out=a_o[:, :], in_=a_sb[:, :])
            nc.sync.dma_start(out=misc_o[:, :], in_=misc[:, :])

    nc.compile()
    return nc


def _get_compiled():
    global _compiled
    if _compiled is None:
        _compiled = _build_bass()
    return _compiled


def _np_softplus(x):
    return np.logaddexp(x, 0.0)


def kernel(y, q_mu, q_log_sigma, z, noise_raw, alpha, variance, _trace=False):
    from concourse.bass_utils import run_bass_kernel_spmd

    nc = _get_compiled()

    f8 = np.float64
    z64 = z.astype(f8)
    al = alpha.astype(f8)
    var = f8(variance[0])
    logvar = np.log(var)

    # z-side stationary blocks (host-built, replicated to all cores).
    # psi2 is symmetric in (i, j): ship only the 2080 upper-tri pairs.
    iu, ju = np.triu_indices(M)                             # (2080,)
    npairs = iu.shape[0]
    Su = z64[iu] + z64[ju]                                  # (2080, q)
    sqz = (z64[:, None, :] - z64[None, :, :]) ** 2          # (m, m, q)
    s1 = 0.25 * (sqz @ al)                                  # (m, m)
    zl = np.zeros((34, 17 * 128), np.float32)
    zl[0:16, :npairs] = Su.T
    zl[16:32, :npairs] = (-0.25 * Su * Su).T
    zl[32, :npairs] = 1.0
    zl[33, :npairs] = -s1[iu, ju]

    zt = z64.T                                              # (q, m)
    zs1 = np.empty((33, M), np.float32)
    zs1[0:16] = zt
    zs1[16:32] = -0.5 * zt * zt
    zs1[32] = 1.0

    consts = np.tile(np.array([[2.0 * logvar, 4.0 * logvar, 0.0, 0.0]],
                              np.float32), (128, 1))
    alpha_in = np.tile(alpha.reshape(1, Q).astype(np.float32), (128, 1))

    in_maps = []
    for i in range(NCORES):
        sl = slice(i * NLOC, (i + 1) * NLOC)
        in_maps.append({
            "y": np.ascontiguousarray(y[sl], dtype=np.float32),
            "qmu": np.ascontiguousarray(q_mu[sl], dtype=np.float32),
            "qls": np.ascontiguousarray(q_log_sigma[sl], dtype=np.float32),
            "zl": zl,
            "zs1": zs1,
            "alpha": alpha_in,
            "consts": consts,
        })

    br = run_bass_kernel_spmd(nc, in_maps, list(range(NCORES)), trace=_trace)
    res = br.results

    psi2_part = np.zeros((128, 17), f8)
    A = np.zeros((M, D), f8)
    misc = np.zeros(8, f8)
    for r in res:
        psi2_part += r["out_psi2"].astype(f8)
        A += r["out_A"].astype(f8)
        misc += r["out_misc"].astype(f8).sum(axis=0)

    flat = psi2_part.T.reshape(17 * 128)
    psi2 = np.empty((M, M), f8)
    psi2[iu, ju] = flat[:npairs]
    psi2[ju, iu] = flat[:npairs]
    lnsig = misc[0] + misc[1]
    ssq = misc[2] + misc[3]
    musq = misc[4] + misc[5]
    tr_yy = misc[6] + misc[7]

    kl_sum = -lnsig + 0.5 * (ssq + musq) - 0.5 * N * Q
    kl_term = kl_sum / (N * D)

    # small m x m algebra on host
    k_mm = var * np.exp(-0.5 * (sqz @ al))                  # (m, m)
    noise_var = _np_softplus(f8(noise_raw[0]))
    beta = 1.0 / noise_var
    psi0 = N * var

    cov1 = beta * psi2 + k_mm
    B = np.linalg.solve(cov1, A)
    tr_yWy = beta * tr_yy - np.sum(A * B)

    F = 0.5 * N * np.log(beta)
    F += 0.5 * np.linalg.slogdet(k_mm)[1]
    F -= 0.5 * N * np.log(np.pi)
    F -= 0.5 * np.linalg.slogdet(cov1)[1]
    F -= 0.5 * beta * psi0
    F += 0.5 * np.trace(np.linalg.solve(k_mm, psi2))
    F = (F * D - 0.5 * tr_yWy) / (N * D)

    out = F - kl_term
    result = np.asarray(out, dtype=np.float32)
    if _trace:
        return result, br
    return result


# revision 16
# speedup vs baseline: 2.0750x; 1.0491x over previous
"""Bayesian GPLVM collapsed-ELBO kernel for Trainium2 (8 NeuronCores).

Sharding: data-parallel over n (2048 rows -> 256 per core). Each core
computes its partial psi2 = sum_n exp(log_psi2_n) (m*m = 4096 entries),
partial A = psi1^T y (64x256), and partial row statistics (KL pieces,
sum y^2). Host sums the 8 partials and does the small m x m linear
algebra (Cholesky solves / slogdets) to produce the scalar ELBO.

Device layout per core (n_loc = 256, two 128-row chunks):
  - NPREP (98 x 256, q-major n-side): rows 0..15 = (q_mu*w1)^T,
    16..31 = w1^T, 32 = h1, 64..79 = (q_mu*w2)^T, 80..95 = w2^T,
    96 = g, 97 = ones (matmul operands need base partition in
    {0,32,64}, so the psi2 block sits at 64). Built n-major as a
    (128 x 98) tile per chunk, then PE-transposed.
  - psi1 exponent = NPREP[0:33,chunk]^T @ ZS1 (z-side, host-built),
    one matmul + Exp per chunk; A accumulates psi1^T y in PSUM.
  - psi2 exponent for each of 32 ij-chunks (128 ij-pairs each) =
    ZL[64:98, chunk]^T @ NPREP[64:98]; Exp with fused free-axis
    accumulation gives the local n-sum directly.
"""

import numpy as np

N, D, Q, M = 2048, 256, 16, 64
NCORES = 8
NLOC = N // NCORES          # 256
F32 = None                  # set lazily (mybir.dt.float32)

_compiled = None


def _build_bass():
    import concourse.bacc as bacc
    import concourse.bass as bass  # noqa: F401
    import concourse.mybir as mybir
    from concourse import masks
    from concourse.tile import TileContext

    f32 = mybir.dt.float32
    f32r = mybir.dt.float32r
    AF = mybir.ActivationFunctionType
    OP = mybir.AluOpType

    nc = bacc.Bacc("TRN2", target_bir_lowering=False, num_swdge_queues=2)

    y_d = nc.declare_dram_parameter("y", [NLOC, D], f32r, isOutput=False)
    qmu_d = nc.declare_dram_parameter("qmu", [NLOC, Q], f32, isOutput=False)
    qls_d = nc.declare_dram_parameter("qls", [NLOC, Q], f32, isOutput=False)
    zl_d = nc.declare_dram_parameter("zl", [34, 17 * 128], f32r, isOutput=False)
    zs1_d = nc.declare_dram_parameter("zs1", [33, M], f32r, isOutput=False)
    alpha_d = nc.declare_dram_parameter("alpha", [128, Q], f32, isOutput=False)
    consts_d = nc.declare_dram_parameter("consts", [128, 4], f32, isOutput=False)
    psi2_o = nc.declare_dram_parameter("out_psi2", [128, 17], f32, isOutput=True)
    a_o = nc.declare_dram_parameter("out_A", [M, D], f32, isOutput=True)
    misc_o = nc.declare_dram_parameter("out_misc", [128, 8], f32, isOutput=True)

    with TileContext(nc) as tc:
        with (
            tc.tile_pool(name="const", bufs=1) as cpool,
            tc.tile_pool(name="big", bufs=1) as bigpool,
            tc.tile_pool(name="work", bufs=3) as wpool,
            tc.tile_pool(name="scr", bufs=3) as spool,
            tc.tile_pool(name="psum", bufs=2, space="PSUM") as ppool,
            tc.tile_pool(name="psums", bufs=1, space="PSUM") as ppools,
            tc.tile_pool(name="psum1", bufs=1, space="PSUM") as ppool1,
        ):
            ident = cpool.tile([128, 128], f32)
            masks.make_identity(nc, ident[:])

            alpha_b = cpool.tile([128, Q], f32)
            nc.sync.dma_start(out=alpha_b[:, :], in_=alpha_d[:, :])

            consts_b = cpool.tile([128, 4], f32)
            nc.sync.dma_start(out=consts_b[:, :], in_=consts_d[:, :])

            zl_sb = bigpool.tile([98, 17 * 128], f32r)
            zs1_sb = cpool.tile([33, M], f32r)
            nc.sync.dma_start(out=zs1_sb[:, :], in_=zs1_d[:, :])

            nprep = bigpool.tile([98, 2 * 128], f32r)
            stats = bigpool.tile([128, 17], f32)
            misc = bigpool.tile([128, 8], f32)
            apsum = ppool1.tile([M, D], f32)

            for c in range(2):
                r0, r1 = c * 128, (c + 1) * 128
                qm = wpool.tile([128, Q], f32, tag="qm")
                nc.sync.dma_start(out=qm[:, :], in_=qmu_d[r0:r1, :])
                qls = wpool.tile([128, Q], f32, tag="qls")
                nc.sync.dma_start(out=qls[:, :], in_=qls_d[r0:r1, :])
                yc = wpool.tile([128, D], f32r, tag="yc")
                nc.sync.dma_start(out=yc[:, :], in_=y_d[r0:r1, :])

                prepn = wpool.tile([128, 98], f32, tag="prepn")
                qsig = wpool.tile([128, Q], f32, tag="qsig")
                d1 = wpool.tile([128, Q], f32, tag="d1")
                d2 = wpool.tile([128, Q], f32, tag="d2")
                rcp = wpool.tile([128, Q], f32, tag="rcp")
                scr16 = spool.tile([128, Q], f32, tag="scr16")
                scrY = spool.tile([128, D], f32, tag="scrY")
                cols = wpool.tile([128, 8], f32, tag="cols")
                sum2c = cols[:, 0:1]
                s3x2c = cols[:, 1:2]
                rt1c = cols[:, 2:3]
                ac = cols[:, 3:4]
                t1c = cols[:, 4:5]
                t2c = cols[:, 5:6]

                # q_sigma = softplus(qls) = ln(1 + exp(qls))
                nc.scalar.activation(scr16[:, :], qls[:, :], AF.Exp)
                nc.scalar.activation(qsig[:, :], scr16[:, :], AF.Ln, bias=1.0)
                nc.vector.tensor_mul(d1[:, :], qsig[:, :], alpha_b[:, :])
                nc.vector.tensor_scalar_add(d1[:, :], d1[:, :], 1.0)
                # w1 = alpha / d1
                nc.vector.reciprocal(rcp[:, :], d1[:, :])
                nc.vector.tensor_mul(prepn[:, 16:32], rcp[:, :], alpha_b[:, :])
                # sum2 = sum_q log d1
                nc.scalar.activation(scr16[:, :], d1[:, :], AF.Ln, accum_out=sum2c)
                # d2 = 2*d1 - 1;  w2 = alpha / d2
                nc.vector.tensor_scalar(
                    out=d2[:, :], in0=d1[:, :], scalar1=2.0, scalar2=-1.0,
                    op0=OP.mult, op1=OP.add)
                nc.vector.reciprocal(rcp[:, :], d2[:, :])
                nc.vector.tensor_mul(prepn[:, 80:96], rcp[:, :], alpha_b[:, :])
                # 2*s3 = sum_q log d2
                nc.scalar.activation(scr16[:, :], d2[:, :], AF.Ln, accum_out=s3x2c)
                # q_mu * w1, q_mu * w2
                nc.vector.tensor_mul(prepn[:, 0:16], qm[:, :], prepn[:, 16:32])
                nc.vector.tensor_mul(prepn[:, 64:80], qm[:, :], prepn[:, 80:96])
                # rt1 = sum_q q_mu^2 w1 ; a = sum_q q_mu^2 w2
                nc.vector.tensor_mul(scr16[:, :], prepn[:, 0:16], qm[:, :])
                nc.vector.tensor_reduce(rt1c, scr16[:, :],
                                        axis=mybir.AxisListType.X, op=OP.add)
                nc.vector.tensor_mul(scr16[:, :], prepn[:, 64:80], qm[:, :])
                nc.vector.tensor_reduce(ac, scr16[:, :],
                                        axis=mybir.AxisListType.X, op=OP.add)
                # h1 = 2*logvar - 0.5*(rt1 + sum2)
                nc.vector.tensor_add(t1c, rt1c, sum2c)
                nc.vector.tensor_scalar(
                    out=prepn[:, 32:33], in0=t1c, scalar1=-0.5,
                    scalar2=consts_b[:, 0:1], op0=OP.mult, op1=OP.add)
                # g = 4*logvar - 0.5*(2*s3) - a
                nc.vector.tensor_scalar(
                    out=t2c, in0=s3x2c, scalar1=0.5, scalar2=ac,
                    op0=OP.mult, op1=OP.add)
                nc.vector.tensor_scalar(
                    out=prepn[:, 96:97], in0=t2c, scalar1=-1.0,
                    scalar2=consts_b[:, 1:2], op0=OP.mult, op1=OP.add)
                nc.vector.memset(prepn[:, 97:98], 1.0)
                nc.vector.memset(prepn[:, 33:64], 0.0)

                # KL / trace statistics (squares on DVE, Ln stays on ACT)
                nc.scalar.activation(scr16[:, :], qsig[:, :], AF.Ln,
                                     accum_out=misc[:, 0 + c:1 + c])
                nc.vector.tensor_mul(scr16[:, :], qsig[:, :], qsig[:, :])
                nc.vector.tensor_reduce(misc[:, 2 + c:3 + c], scr16[:, :],
                                        axis=mybir.AxisListType.X, op=OP.add)
                nc.vector.tensor_mul(scr16[:, :], qm[:, :], qm[:, :])
                nc.vector.tensor_reduce(misc[:, 4 + c:5 + c], scr16[:, :],
                                        axis=mybir.AxisListType.X, op=OP.add)
                nc.vector.tensor_mul(scrY[:, :], yc[:, :].bitcast(f32), yc[:, :].bitcast(f32))
                nc.vector.tensor_reduce(misc[:, 6 + c:7 + c], scrY[:, :],
                                        axis=mybir.AxisListType.X, op=OP.add)

                # transpose prep (128 x 67) -> NPREP[:, chunk]
                ptp = ppools.tile([98, 128], f32, tag="ptp")
                nc.tensor.transpose(ptp[:, :], prepn[:, :], ident[:, :])
                nc.vector.tensor_copy(nprep[:, r0:r1], ptp[:, :])

                # psi1 chunk: exponent (128 n x 64 m) then exp
                e1 = ppools.tile([128, M], f32, tag="e1")
                nc.tensor.matmul(e1[:, :],
                                 lhsT=nprep[0:33, r0:r1],
                                 rhs=zs1_sb[:, :],
                                 start=True, stop=True)
                psi1c = wpool.tile([128, M], f32r, tag="psi1c")
                nc.scalar.activation(psi1c[:, :], e1[:, :], AF.Exp)
                # A += psi1_c^T @ y_c
                nc.tensor.matmul(apsum[:, :], lhsT=psi1c[:, :],
                                 rhs=yc[:, :],
                                 start=(c == 0), stop=(c == 1))

            # zl is big (~290KB): issue after the small DMAs in sync-engine
            # program order (ring completes in order), split per chunk-group
            # so each psi2 group starts as soon as its slice lands
            for t in range(5):
                c0, c1 = t * 512, min((t + 1) * 512, 17 * 128)
                nc.sync.dma_start(out=zl_sb[64:98, c0:c1], in_=zl_d[:, c0:c1])

            # psi2 is symmetric: only the 2080 upper-triangle ij-pairs
            # (17 chunks of 128, last 96 slots are padding), 4 chunks per
            # PSUM tile
            for t in range(5):
                nch = min(4, 17 - 4 * t)
                p2 = ppool.tile([128, 4 * NLOC], f32, tag="p2")
                for j in range(nch):
                    ch = 4 * t + j
                    nc.tensor.matmul(
                        p2[:, j * NLOC:(j + 1) * NLOC],
                        lhsT=zl_sb[64:98, ch * 128:(ch + 1) * 128],
                        rhs=nprep[64:98, :],
                        start=True, stop=True)
                scr = spool.tile([128, 4 * NLOC], f32, tag="p2scr")
                w = nch * NLOC
                nc.scalar.activation(scr[:, :w], p2[:, :w], AF.Exp)
                nc.vector.tensor_reduce(
                    stats[:, 4 * t:4 * t + nch],
                    scr[:, :w].rearrange("p (a b) -> p a b", b=NLOC),
                    axis=mybir.AxisListType.X, op=OP.add)

            a_sb = bigpool.tile([M, D], f32)
            nc.vector.tensor_copy(a_sb[:, :], apsum[:, :])
            nc.sync.dma_start(out=psi2_o[:, :], in_=stats[:, :])
            nc.sync.dma_start(out=a_o[:, :], in_=a_sb[:, :])
            nc.sync.dma_start(out=misc_o[:, :], in_=misc[:, :])

    nc.compile()
    return nc


def _get_compiled():
    global _compiled
    if _compiled is None:
        _compiled = _build_bass()
    return _compiled


def _np_softplus(x):
    return np.logaddexp(x, 0.0)


def kernel(y, q_mu, q_log_sigma, z, noise_raw, alpha, variance, _trace=False):
    from concourse.bass_utils import run_bass_kernel_spmd

    nc = _get_compiled()

    f8 = np.float64
    z64 = z.astype(f8)
    al = alpha.astype(f8)
    var = f8(variance[0])
    logvar = np.log(var)

    # z-side stationary blocks (host-built, replicated to all cores).
    # psi2 is symmetric in (i, j): ship only the 2080 upper-tri pairs.
    iu, ju = np.triu_indices(M)                             # (2080,)
    npairs = iu.shape[0]
    Su = z64[iu] + z64[ju]                                  # (2080, q)
    sqz = (z64[:, None, :] - z64[None, :, :]) ** 2          # (m, m, q)
    s1 = 0.25 * (sqz @ al)                                  # (m, m)
    zl = np.zeros((34, 17 * 128), np.float32)
    zl[0:16, :npairs] = Su.T
    zl[16:32, :npairs] = (-0.25 * Su * Su).T
    zl[32, :npairs] = 1.0
    zl[33, :npairs] = -s1[iu, ju]

    zt = z64.T                                              # (q, m)
    zs1 = np.empty((33, M), np.float32)
    zs1[0:16] = zt
    zs1[16:32] = -0.5 * zt * zt
    zs1[32] = 1.0

    consts = np.tile(np.array([[2.0 * logvar, 4.0 * logvar, 0.0, 0.0]],
                              np.float32), (128, 1))
    alpha_in = np.tile(alpha.reshape(1, Q).astype(np.float32), (128, 1))

    in_maps = []
    for i in range(NCORES):
        sl = slice(i * NLOC, (i + 1) * NLOC)
        in_maps.append({
            "y": np.ascontiguousarray(y[sl], dtype=np.float32),
            "qmu": np.ascontiguousarray(q_mu[sl], dtype=np.float32),
            "qls": np.ascontiguousarray(q_log_sigma[sl], dtype=np.float32),
            "zl": zl,
            "zs1": zs1,
            "alpha": alpha_in,
            "consts": consts,
        })

    br = run_bass_kernel_spmd(nc, in_maps, list(range(NCORES)), trace=_trace)
    res = br.results

    psi2_part = np.zeros((128, 17), f8)
    A = np.zeros((M, D), f8)
    misc = np.zeros(8, f8)
    for r in res:
        psi2_part += r["out_psi2"].astype(f8)
        A += r["out_A"].astype(f8)
        misc += r["out_misc"].astype(f8).sum(axis=0)

    flat = psi2_part.T.reshape(17 * 128)
    psi2 = np.empty((M, M), f8)
    psi2[iu, ju] = flat[:npairs]
    psi2[ju, iu] = flat[:npairs]
    lnsig = misc[0] + misc[1]
    ssq = misc[2] + misc[3]
    musq = misc[4] + misc[5]
    tr_yy = misc[6] + misc[7]

    kl_sum = -lnsig + 0.5 * (ssq + musq) - 0.5 * N * Q
    kl_term = kl_sum / (N * D)

    # small m x m algebra on host
    k_mm = var * np.exp(-0.5 * (sqz @ al))                  # (m, m)
    noise_var = _np_softplus(f8(noise_raw[0]))
    beta = 1.0 / noise_var
    psi0 = N * var

    cov1 = beta * psi2 + k_mm
    B = np.linalg.solve(cov1, A)
    tr_yWy = beta * tr_yy - np.sum(A * B)

    F = 0.5 * N * np.log(beta)
    F += 0.5 * np.linalg.slogdet(k_mm)[1]
    F -= 0.5 * N * np.log(np.pi)
    F -= 0.5 * np.linalg.slogdet(cov1)[1]
    F -= 0.5 * beta * psi0
    F += 0.5 * np.trace(np.linalg.solve(k_mm, psi2))
    F = (F * D - 0.5 * tr_yWy) / (N * D)

    out = F - kl_term
    result = np.asarray(out, dtype=np.float32)
    if _trace:
        return result, br
    return result


# revision 17
# speedup vs baseline: 2.3341x; 1.1249x over previous
"""Bayesian GPLVM collapsed-ELBO kernel for Trainium2 (8 NeuronCores).

Sharding: data-parallel over n (2048 rows -> 256 per core). Each core
computes its partial psi2 = sum_n exp(log_psi2_n) (m*m = 4096 entries),
partial A = psi1^T y (64x256), and partial row statistics (KL pieces,
sum y^2). Host sums the 8 partials and does the small m x m linear
algebra (Cholesky solves / slogdets) to produce the scalar ELBO.

Device layout per core (n_loc = 256, two 128-row chunks):
  - NPREP (98 x 256, q-major n-side): rows 0..15 = (q_mu*w1)^T,
    16..31 = w1^T, 32 = h1, 64..79 = (q_mu*w2)^T, 80..95 = w2^T,
    96 = g, 97 = ones (matmul operands need base partition in
    {0,32,64}, so the psi2 block sits at 64). Built n-major as a
    (128 x 98) tile per chunk, then PE-transposed.
  - psi1 exponent = NPREP[0:33,chunk]^T @ ZS1 (z-side, host-built),
    one matmul + Exp per chunk; A accumulates psi1^T y in PSUM.
  - psi2 exponent for each of 32 ij-chunks (128 ij-pairs each) =
    ZL[64:98, chunk]^T @ NPREP[64:98]; Exp with fused free-axis
    accumulation gives the local n-sum directly.
"""

import numpy as np

N, D, Q, M = 2048, 256, 16, 64
NCORES = 8
NLOC = N // NCORES          # 256
F32 = None                  # set lazily (mybir.dt.float32)

_compiled = None


def _build_bass():
    import concourse.bacc as bacc
    import concourse.bass as bass  # noqa: F401
    import concourse.mybir as mybir
    from concourse import masks
    from concourse.tile import TileContext

    f32 = mybir.dt.float32
    f32r = mybir.dt.float32r
    AF = mybir.ActivationFunctionType
    OP = mybir.AluOpType

    nc = bacc.Bacc("TRN2", target_bir_lowering=False, num_swdge_queues=2)

    y_d = nc.declare_dram_parameter("y", [NLOC, D], f32r, isOutput=False)
    qin_d = nc.declare_dram_parameter("qin", [NLOC, 2 * Q], f32, isOutput=False)
    zl_d = nc.declare_dram_parameter("zl", [34, 17 * 128], f32r, isOutput=False)
    zs1_d = nc.declare_dram_parameter("zs1", [33, M], f32r, isOutput=False)
    acon_d = nc.declare_dram_parameter("acon", [128, Q + 4], f32, isOutput=False)
    psi2_o = nc.declare_dram_parameter("out_psi2", [128, 17], f32, isOutput=True)
    a_o = nc.declare_dram_parameter("out_A", [M, D], f32, isOutput=True)
    misc_o = nc.declare_dram_parameter("out_misc", [128, 8], f32, isOutput=True)

    with TileContext(nc) as tc:
        with (
            tc.tile_pool(name="const", bufs=1) as cpool,
            tc.tile_pool(name="big", bufs=1) as bigpool,
            tc.tile_pool(name="work", bufs=3) as wpool,
            tc.tile_pool(name="scr", bufs=3) as spool,
            tc.tile_pool(name="psum", bufs=2, space="PSUM") as ppool,
            tc.tile_pool(name="psums", bufs=1, space="PSUM") as ppools,
            tc.tile_pool(name="psum1", bufs=1, space="PSUM") as ppool1,
        ):
            ident = cpool.tile([128, 128], f32)
            masks.make_identity(nc, ident[:])

            acon = cpool.tile([128, Q + 4], f32)
            nc.sync.dma_start(out=acon[:, :], in_=acon_d[:, :])
            alpha_b = acon[:, 0:Q]
            consts_b = acon[:, Q:Q + 4]

            zl_sb = bigpool.tile([98, 17 * 128], f32r)
            zs1_sb = cpool.tile([33, M], f32r)

            nprep = bigpool.tile([98, 2 * 128], f32r)
            stats = bigpool.tile([128, 17], f32)
            misc = bigpool.tile([128, 8], f32)
            apsum = ppool1.tile([M, D], f32)

            zs1_loaded = False
            for c in range(2):
                r0, r1 = c * 128, (c + 1) * 128
                qin = wpool.tile([128, 2 * Q], f32, tag="qin")
                nc.sync.dma_start(out=qin[:, :], in_=qin_d[r0:r1, :])
                qm = qin[:, 0:Q]
                qls = qin[:, Q:2 * Q]
                yc = wpool.tile([128, D], f32r, tag="yc")
                nc.sync.dma_start(out=yc[:, :], in_=y_d[r0:r1, :])
                if not zs1_loaded:
                    nc.sync.dma_start(out=zs1_sb[:, :], in_=zs1_d[:, :])
                    zs1_loaded = True

                prepn = wpool.tile([128, 98], f32, tag="prepn")
                qsig = wpool.tile([128, Q], f32, tag="qsig")
                d1 = wpool.tile([128, Q], f32, tag="d1")
                d2 = wpool.tile([128, Q], f32, tag="d2")
                rcp = wpool.tile([128, Q], f32, tag="rcp")
                scr16 = spool.tile([128, Q], f32, tag="scr16")
                scrY = spool.tile([128, D], f32, tag="scrY")
                cols = wpool.tile([128, 8], f32, tag="cols")
                sum2c = cols[:, 0:1]
                s3x2c = cols[:, 1:2]
                rt1c = cols[:, 2:3]
                ac = cols[:, 3:4]
                t1c = cols[:, 4:5]
                t2c = cols[:, 5:6]

                # q_sigma = softplus(qls) = ln(1 + exp(qls))
                nc.scalar.activation(scr16[:, :], qls, AF.Exp)
                nc.scalar.activation(qsig[:, :], scr16[:, :], AF.Ln, bias=1.0)
                nc.vector.tensor_mul(d1[:, :], qsig[:, :], alpha_b)
                nc.vector.tensor_scalar_add(d1[:, :], d1[:, :], 1.0)
                # w1 = alpha / d1
                nc.vector.reciprocal(rcp[:, :], d1[:, :])
                nc.vector.tensor_mul(prepn[:, 16:32], rcp[:, :], alpha_b)
                # sum2 = sum_q log d1
                nc.scalar.activation(scr16[:, :], d1[:, :], AF.Ln, accum_out=sum2c)
                # d2 = 2*d1 - 1;  w2 = alpha / d2
                nc.vector.tensor_scalar(
                    out=d2[:, :], in0=d1[:, :], scalar1=2.0, scalar2=-1.0,
                    op0=OP.mult, op1=OP.add)
                nc.vector.reciprocal(rcp[:, :], d2[:, :])
                nc.vector.tensor_mul(prepn[:, 80:96], rcp[:, :], alpha_b)
                # 2*s3 = sum_q log d2
                nc.scalar.activation(scr16[:, :], d2[:, :], AF.Ln, accum_out=s3x2c)
                # q_mu * w1, q_mu * w2
                nc.vector.tensor_mul(prepn[:, 0:16], qm, prepn[:, 16:32])
                nc.vector.tensor_mul(prepn[:, 64:80], qm, prepn[:, 80:96])
                # rt1 = sum_q q_mu^2 w1 ; a = sum_q q_mu^2 w2
                nc.vector.tensor_mul(scr16[:, :], prepn[:, 0:16], qm)
                nc.vector.tensor_reduce(rt1c, scr16[:, :],
                                        axis=mybir.AxisListType.X, op=OP.add)
                nc.vector.tensor_mul(scr16[:, :], prepn[:, 64:80], qm)
                nc.vector.tensor_reduce(ac, scr16[:, :],
                                        axis=mybir.AxisListType.X, op=OP.add)
                # h1 = 2*logvar - 0.5*(rt1 + sum2)
                nc.vector.tensor_add(t1c, rt1c, sum2c)
                nc.vector.tensor_scalar(
                    out=prepn[:, 32:33], in0=t1c, scalar1=-0.5,
                    scalar2=consts_b[:, 0:1], op0=OP.mult, op1=OP.add)
                # g = 4*logvar - 0.5*(2*s3) - a
                nc.vector.tensor_scalar(
                    out=t2c, in0=s3x2c, scalar1=0.5, scalar2=ac,
                    op0=OP.mult, op1=OP.add)
                nc.vector.tensor_scalar(
                    out=prepn[:, 96:97], in0=t2c, scalar1=-1.0,
                    scalar2=consts_b[:, 1:2], op0=OP.mult, op1=OP.add)
                nc.vector.memset(prepn[:, 97:98], 1.0)
                nc.vector.memset(prepn[:, 33:64], 0.0)

                # KL / trace statistics (squares on DVE, Ln stays on ACT)
                nc.scalar.activation(scr16[:, :], qsig[:, :], AF.Ln,
                                     accum_out=misc[:, 0 + c:1 + c])
                nc.vector.tensor_mul(scr16[:, :], qsig[:, :], qsig[:, :])
                nc.vector.tensor_reduce(misc[:, 2 + c:3 + c], scr16[:, :],
                                        axis=mybir.AxisListType.X, op=OP.add)
                nc.vector.tensor_mul(scr16[:, :], qm, qm)
                nc.vector.tensor_reduce(misc[:, 4 + c:5 + c], scr16[:, :],
                                        axis=mybir.AxisListType.X, op=OP.add)
                nc.vector.tensor_mul(scrY[:, :], yc[:, :].bitcast(f32), yc[:, :].bitcast(f32))
                nc.vector.tensor_reduce(misc[:, 6 + c:7 + c], scrY[:, :],
                                        axis=mybir.AxisListType.X, op=OP.add)

                # transpose prep (128 x 67) -> NPREP[:, chunk]
                ptp = ppools.tile([98, 128], f32, tag="ptp")
                nc.tensor.transpose(ptp[:, :], prepn[:, :], ident[:, :])
                nc.vector.tensor_copy(nprep[:, r0:r1], ptp[:, :])

                # psi1 chunk: exponent (128 n x 64 m) then exp
                e1 = ppools.tile([128, M], f32, tag="e1")
                nc.tensor.matmul(e1[:, :],
                                 lhsT=nprep[0:33, r0:r1],
                                 rhs=zs1_sb[:, :],
                                 start=True, stop=True)
                psi1c = wpool.tile([128, M], f32r, tag="psi1c")
                nc.scalar.activation(psi1c[:, :], e1[:, :], AF.Exp)
                # A += psi1_c^T @ y_c
                nc.tensor.matmul(apsum[:, :], lhsT=psi1c[:, :],
                                 rhs=yc[:, :],
                                 start=(c == 0), stop=(c == 1))

            # zl is big (~290KB): issue after the small DMAs in sync-engine
            # program order (ring completes in order), split per chunk-group
            # so each psi2 group starts as soon as its slice lands
            for t in range(5):
                c0, c1 = t * 512, min((t + 1) * 512, 17 * 128)
                nc.sync.dma_start(out=zl_sb[64:98, c0:c1], in_=zl_d[:, c0:c1])

            # psi2 is symmetric: only the 2080 upper-triangle ij-pairs
            # (17 chunks of 128, last 96 slots are padding), 4 chunks per
            # PSUM tile
            for t in range(5):
                nch = min(4, 17 - 4 * t)
                p2 = ppool.tile([128, 4 * NLOC], f32, tag="p2")
                for j in range(nch):
                    ch = 4 * t + j
                    nc.tensor.matmul(
                        p2[:, j * NLOC:(j + 1) * NLOC],
                        lhsT=zl_sb[64:98, ch * 128:(ch + 1) * 128],
                        rhs=nprep[64:98, :],
                        start=True, stop=True)
                scr = spool.tile([128, 4 * NLOC], f32, tag="p2scr")
                w = nch * NLOC
                nc.scalar.activation(scr[:, :w], p2[:, :w], AF.Exp)
                nc.vector.tensor_reduce(
                    stats[:, 4 * t:4 * t + nch],
                    scr[:, :w].rearrange("p (a b) -> p a b", b=NLOC),
                    axis=mybir.AxisListType.X, op=OP.add)

            a_sb = bigpool.tile([M, D], f32)
            nc.vector.tensor_copy(a_sb[:, :], apsum[:, :])
            nc.sync.dma_start(out=psi2_o[:, :], in_=stats[:, :])
            nc.sync.dma_start(out=a_o[:, :], in_=a_sb[:, :])
            nc.sync.dma_start(out=misc_o[:, :], in_=misc[:, :])

    nc.compile()
    return nc


def _get_compiled():
    global _compiled
    if _compiled is None:
        _compiled = _build_bass()
    return _compiled


def _np_softplus(x):
    return np.logaddexp(x, 0.0)


def kernel(y, q_mu, q_log_sigma, z, noise_raw, alpha, variance, _trace=False):
    from concourse.bass_utils import run_bass_kernel_spmd

    nc = _get_compiled()

    f8 = np.float64
    z64 = z.astype(f8)
    al = alpha.astype(f8)
    var = f8(variance[0])
    logvar = np.log(var)

    # z-side stationary blocks (host-built, replicated to all cores).
    # psi2 is symmetric in (i, j): ship only the 2080 upper-tri pairs.
    iu, ju = np.triu_indices(M)                             # (2080,)
    npairs = iu.shape[0]
    Su = z64[iu] + z64[ju]                                  # (2080, q)
    sqz = (z64[:, None, :] - z64[None, :, :]) ** 2          # (m, m, q)
    s1 = 0.25 * (sqz @ al)                                  # (m, m)
    zl = np.zeros((34, 17 * 128), np.float32)
    zl[0:16, :npairs] = Su.T
    zl[16:32, :npairs] = (-0.25 * Su * Su).T
    zl[32, :npairs] = 1.0
    zl[33, :npairs] = -s1[iu, ju]

    zt = z64.T                                              # (q, m)
    zs1 = np.empty((33, M), np.float32)
    zs1[0:16] = zt
    zs1[16:32] = -0.5 * zt * zt
    zs1[32] = 1.0

    acon = np.empty((128, Q + 4), np.float32)
    acon[:, 0:Q] = alpha.reshape(1, Q).astype(np.float32)
    acon[:, Q:Q + 4] = np.array([2.0 * logvar, 4.0 * logvar, 0.0, 0.0],
                                np.float32)
    qin_full = np.concatenate(
        [q_mu.astype(np.float32), q_log_sigma.astype(np.float32)], axis=1)

    in_maps = []
    for i in range(NCORES):
        sl = slice(i * NLOC, (i + 1) * NLOC)
        in_maps.append({
            "y": np.ascontiguousarray(y[sl], dtype=np.float32),
            "qin": np.ascontiguousarray(qin_full[sl]),
            "zl": zl,
            "zs1": zs1,
            "acon": acon,
        })

    br = run_bass_kernel_spmd(nc, in_maps, list(range(NCORES)), trace=_trace)
    res = br.results

    psi2_part = np.zeros((128, 17), f8)
    A = np.zeros((M, D), f8)
    misc = np.zeros(8, f8)
    for r in res:
        psi2_part += r["out_psi2"].astype(f8)
        A += r["out_A"].astype(f8)
        misc += r["out_misc"].astype(f8).sum(axis=0)

    flat = psi2_part.T.reshape(17 * 128)
    psi2 = np.empty((M, M), f8)
    psi2[iu, ju] = flat[:npairs]
    psi2[ju, iu] = flat[:npairs]
    lnsig = misc[0] + misc[1]
    ssq = misc[2] + misc[3]
    musq = misc[4] + misc[5]
    tr_yy = misc[6] + misc[7]

    kl_sum = -lnsig + 0.5 * (ssq + musq) - 0.5 * N * Q
    kl_term = kl_sum / (N * D)

    # small m x m algebra on host
    k_mm = var * np.exp(-0.5 * (sqz @ al))                  # (m, m)
    noise_var = _np_softplus(f8(noise_raw[0]))
    beta = 1.0 / noise_var
    psi0 = N * var

    cov1 = beta * psi2 + k_mm
    B = np.linalg.solve(cov1, A)
    tr_yWy = beta * tr_yy - np.sum(A * B)

    F = 0.5 * N * np.log(beta)
    F += 0.5 * np.linalg.slogdet(k_mm)[1]
    F -= 0.5 * N * np.log(np.pi)
    F -= 0.5 * np.linalg.slogdet(cov1)[1]
    F -= 0.5 * beta * psi0
    F += 0.5 * np.trace(np.linalg.solve(k_mm, psi2))
    F = (F * D - 0.5 * tr_yWy) / (N * D)

    out = F - kl_term
    result = np.asarray(out, dtype=np.float32)
    if _trace:
        return result, br
    return result
